# revision 1
# baseline (speedup 1.0000x reference)
"""Trainium2 Bass kernel for nn_NeuralEncoder (sparse banded attention encoder).

Sharding: 8 cores = (batch b in 0..3) x (sequence half h in 0..1), zero
collectives. Uniform SPMD program over a 1024-row local window per core:
h=0 cores get 512 zero-pad rows + rows 0..511, h=1 cores get rows 0..1023.
Each layer shrinks the active window by 128 rows at the front (the
CB=128 sliding-window halo); every core emits local rows 512..1023 as its
512 output rows.

Numerics: bf16 matmuls with fp32 PSUM accumulation; LayerNorm, softmax and
the residual stream in fp32. LN gains are folded into the following weight
matrices host-side; the band/padding/spikes_mask is a host-precomputed
additive bias applied to attention scores pre-exp.
"""

import os
import sys

for _p in ("/opt/trn_rl_repo", "/root/.axon_site/_ro/trn_rl_repo"):
    if _p not in sys.path and os.path.isdir(_p):
        sys.path.append(_p)

import numpy as np
import ml_dtypes

from concourse import bacc
import concourse.tile as tile
from concourse import mybir
from concourse.bass_utils import run_bass_kernel_spmd
from concourse.masks import make_identity

# dims
B, T, C, D, H, NH, HD, INTER, L = 4, 1024, 256, 256, 512, 8, 64, 2048, 4
CF, CB, BASE = 0, 128, 10000.0
P = 128
NB = T // P          # 8 local row blocks
N_CORES = 8
NEG = np.float32(-1e30)
F32 = mybir.dt.float32
BF16 = mybir.dt.bfloat16
AF = mybir.ActivationFunctionType

_PROG_CACHE = {}


def _spans(start_block, end_block, max_blocks=4):
    """Split block range [start_block, end_block) into runs of <= max_blocks."""
    out = []
    b = start_block
    while b < end_block:
        e = min(b + max_blocks, end_block)
        out.append((b, e))
        b = e
    return out


def _build_program(has_bias):
    nc = bacc.Bacc("TRN2", target_bir_lowering=False, debug=False,
                   num_devices=N_CORES)

    # ---- DRAM I/O ----
    d_spikesT = nc.dram_tensor("spikesT", [C, T], BF16, kind="ExternalInput")
    d_csT = nc.dram_tensor("csT", [P, T], F32, kind="ExternalInput")
    d_snT = nc.dram_tensor("snT", [P, T], F32, kind="ExternalInput")
    d_maskT = nc.dram_tensor("maskT", [NB, P, 2 * P], F32, kind="ExternalInput")
    d_rotm = nc.dram_tensor("rotm", [P, P], BF16, kind="ExternalInput")
    d_embw = nc.dram_tensor("embw", [C, D], BF16, kind="ExternalInput")
    d_projw = nc.dram_tensor("projw", [D, H], BF16, kind="ExternalInput")
    d_wq, d_wk, d_wv, d_wo, d_upw, d_dnw = [], [], [], [], [], []
    for l in range(L):
        d_wq.append(nc.dram_tensor(f"wq{l}", [H, H], BF16, kind="ExternalInput"))
        d_wk.append(nc.dram_tensor(f"wk{l}", [H, H], BF16, kind="ExternalInput"))
        d_wv.append(nc.dram_tensor(f"wv{l}", [H, H], BF16, kind="ExternalInput"))
        d_wo.append(nc.dram_tensor(f"wo{l}", [H, H], BF16, kind="ExternalInput"))
        d_upw.append(nc.dram_tensor(f"upw{l}", [H, INTER], BF16, kind="ExternalInput"))
        d_dnw.append(nc.dram_tensor(f"dnw{l}", [INTER, H], BF16, kind="ExternalInput"))
    if has_bias:
        d_embb = nc.dram_tensor("embb", [D], F32, kind="ExternalInput")
        d_projb = nc.dram_tensor("projb", [1, H], BF16, kind="ExternalInput")
        d_bq = [nc.dram_tensor(f"bq{l}", [H], F32, kind="ExternalInput") for l in range(L)]
        d_bk = [nc.dram_tensor(f"bk{l}", [H], F32, kind="ExternalInput") for l in range(L)]
        d_bv = [nc.dram_tensor(f"bv{l}", [1, H], BF16, kind="ExternalInput") for l in range(L)]
        d_bo = [nc.dram_tensor(f"bo{l}", [1, H], BF16, kind="ExternalInput") for l in range(L)]
        d_upb = [nc.dram_tensor(f"upb{l}", [INTER], F32, kind="ExternalInput") for l in range(L)]
        d_dnb = [nc.dram_tensor(f"dnb{l}", [1, H], BF16, kind="ExternalInput") for l in range(L)]
    d_out = nc.dram_tensor("out", [T // 2, H], F32, kind="ExternalOutput")

    with tile.TileContext(nc) as tc:
        with (
            tc.tile_pool(name="consts", bufs=1) as consts,
            tc.tile_pool(name="wts", bufs=2) as wts,
            tc.tile_pool(name="work", bufs=2) as work,
            tc.tile_pool(name="small", bufs=6) as small,
            tc.tile_pool(name="hTs", bufs=2) as hTs,
            tc.tile_pool(name="qk", bufs=1) as qk,
            tc.tile_pool(name="vp", bufs=9) as vp,
            tc.tile_pool(name="es", bufs=3) as es,
            tc.tile_pool(name="itp", bufs=1) as itp,
            tc.tile_pool(name="mm_ps", bufs=3, space="PSUM") as mm_ps,
            tc.tile_pool(name="s_ps", bufs=2, space="PSUM") as s_ps,
            tc.tile_pool(name="o_ps", bufs=2, space="PSUM") as o_ps,
            tc.tile_pool(name="t_ps", bufs=1, space="PSUM") as t_ps,
        ):
            # ---- constants ----
            ident = consts.tile([P, P], BF16, tag="ident")
            make_identity(nc, ident[:])
            eps = consts.tile([P, 1], F32, tag="eps")
            nc.vector.memset(eps[:], 1e-5)
            csT = consts.tile([P, T], F32, tag="csT")
            nc.sync.dma_start(out=csT[:], in_=d_csT.ap())
            snT = consts.tile([P, T], F32, tag="snT")
            nc.sync.dma_start(out=snT[:], in_=d_snT.ap())
            maskT = consts.tile([P, NB, 2 * P], F32, tag="maskT")
            nc.sync.dma_start(out=maskT[:], in_=d_maskT.ap().rearrange("k p q -> p k q"))
            spT = consts.tile([P, C // P, T], BF16, tag="spT")
            nc.sync.dma_start(out=spT[:], in_=d_spikesT.ap().rearrange("(c p) r -> p c r", p=P))
            rotm = consts.tile([P, P], BF16, tag="rotm")
            nc.sync.dma_start(out=rotm[:], in_=d_rotm.ap())
            embw = consts.tile([P, C // P, D], BF16, tag="embw")
            nc.sync.dma_start(out=embw[:], in_=d_embw.ap().rearrange("(c p) d -> p c d", p=P))
            projw = consts.tile([P, D // P, H], BF16, tag="projw")
            nc.sync.dma_start(out=projw[:], in_=d_projw.ap().rearrange("(c p) h -> p c h", p=P))
            if has_bias:
                embb = consts.tile([P, D // P], F32, tag="embb")
                nc.sync.dma_start(out=embb[:], in_=d_embb.ap().rearrange("(c p) -> p c", p=P))
                projb = consts.tile([1, H], BF16, tag="projb")
                nc.sync.dma_start(out=projb[:], in_=d_projb.ap())
                ones_r = consts.tile([1, P], BF16, tag="ones_r")
                nc.vector.memset(ones_r[:], 1.0)

            x = consts.tile([P, NB, H], F32, tag="x")
            gT = consts.tile([P, D // P, T], BF16, tag="gT")

            def mm_group(ps, pairs, bias_row=None):
                """Accumulate lhsT.T @ rhs pairs into ps; optional bias row
                (psum += ones^T @ bias_row) closes the group."""
                for i, (a, bb) in enumerate(pairs):
                    last = (i == len(pairs) - 1) and bias_row is None
                    nc.tensor.matmul(ps, a, bb, start=(i == 0), stop=last)
                if bias_row is not None:
                    nc.tensor.matmul(ps, ones_r[:], bias_row,
                                     start=False, stop=True)

            # ---- embedding: gT = gelu(spikes @ embed_w)^T, x = gT^T @ proj_w ----
            for oc in range(D // P):
                for (s0, s1) in _spans(0, NB):
                    n = (s1 - s0) * P
                    ps = mm_ps.tile([P, 512], F32, tag="mm", name="mmps")[:, :n]
                    for fc in range(C // P):
                        nc.tensor.matmul(ps, embw[:, fc, oc * P:(oc + 1) * P],
                                         spT[:, fc, s0 * P:s0 * P + n],
                                         start=(fc == 0), stop=(fc == C // P - 1))
                    bias = embb[:, oc:oc + 1] if has_bias else 0.0
                    nc.scalar.activation(gT[:, oc, s0 * P:s0 * P + n], ps, AF.Gelu,
                                         bias=bias)
            for rb in range(NB):
                ps = mm_ps.tile([P, 512], F32, tag="mm")
                mm_group(ps,
                         [(gT[:, fc, rb * P:(rb + 1) * P], projw[:, fc, :])
                          for fc in range(D // P)],
                         bias_row=projb[:] if has_bias else None)
                nc.scalar.activation(x[:, rb, :], ps, AF.Copy)

            # ---- layers ----
            _trunc = os.environ.get("KTRUNC", "")
            n_layers = L
            if _trunc.startswith("L"):
                n_layers = int(_trunc[1:].split(":")[0])
            _phase = _trunc.split(":")[1] if ":" in _trunc else "all"
            for l in range(n_layers):
                kb0, qb0 = l, l + 1

                wq = wts.tile([P, H // P, H], BF16, tag="wq")
                nc.sync.dma_start(out=wq[:], in_=d_wq[l].ap().rearrange("(f p) o -> p f o", p=P))
                wk = wts.tile([P, H // P, H], BF16, tag="wk")
                nc.sync.dma_start(out=wk[:], in_=d_wk[l].ap().rearrange("(f p) o -> p f o", p=P))
                wv = wts.tile([P, H // P, H], BF16, tag="wv")
                nc.sync.dma_start(out=wv[:], in_=d_wv[l].ap().rearrange("(f p) o -> p f o", p=P))
                wo = wts.tile([P, H // P, H], BF16, tag="wo")
                nc.sync.dma_start(out=wo[:], in_=d_wo[l].ap().rearrange("(f p) o -> p f o", p=P))
                if has_bias:
                    bq = wts.tile([P, H // P], F32, tag="bq")
                    nc.sync.dma_start(out=bq[:], in_=d_bq[l].ap().rearrange("(c p) -> p c", p=P))
                    bk = wts.tile([P, H // P], F32, tag="bk")
                    nc.sync.dma_start(out=bk[:], in_=d_bk[l].ap().rearrange("(c p) -> p c", p=P))
                    bv = wts.tile([1, H], BF16, tag="bv")
                    nc.sync.dma_start(out=bv[:], in_=d_bv[l].ap())
                    bo = wts.tile([1, H], BF16, tag="bo")
                    nc.sync.dma_start(out=bo[:], in_=d_bo[l].ap())
                    dnb = wts.tile([1, H], BF16, tag="dnb")
                    nc.sync.dma_start(out=dnb[:], in_=d_dnb[l].ap())
                    upb = wts.tile([P, INTER // P], F32, tag="upb")
                    nc.sync.dma_start(out=upb[:], in_=d_upb[l].ap().rearrange("(c p) -> p c", p=P))

                def layernorm(src_ap, dst_bf16_ap):
                    stats = small.tile([P, 6], F32, tag="stats")
                    nc.vector.bn_stats(stats[:], src_ap)
                    mv = small.tile([P, 2], F32, tag="mv")
                    nc.vector.bn_aggr(mv[:], stats[:])
                    rstd = small.tile([P, 1], F32, tag="rstd")
                    nc.scalar.activation(rstd[:], mv[:, 1:2], AF.Sqrt, bias=eps[:])
                    nc.vector.reciprocal(rstd[:], rstd[:])
                    nc.vector.tensor_scalar(dst_bf16_ap, src_ap,
                                            mv[:, 0:1], rstd[:],
                                            mybir.AluOpType.subtract,
                                            mybir.AluOpType.mult)

                def transpose128(src_bf16_ap, dst_bf16_ap):
                    # src [128, 128] -> dst [128, 128] via PE transpose
                    tp = t_ps.tile([P, P], BF16, tag="tp")
                    nc.tensor.transpose(tp[:], src_bf16_ap, ident[:])
                    nc.scalar.activation(dst_bf16_ap, tp[:], AF.Copy)

                # LN1 + h^T + v for key range
                hT = hTs.tile([P, H // P, T], BF16, tag="hT")
                vtiles = {}
                for kb in range(kb0, NB):
                    hrow = work.tile([P, H], BF16, tag="hrow")
                    layernorm(x[:, kb, :], hrow[:])
                    for fc in range(H // P):
                        transpose128(hrow[:, fc * P:(fc + 1) * P],
                                     hT[:, fc, kb * P:(kb + 1) * P])
                    ps = mm_ps.tile([P, 512], F32, tag="mm")
                    mm_group(ps,
                             [(hT[:, fc, kb * P:(kb + 1) * P], wv[:, fc, :])
                              for fc in range(H // P)],
                             bias_row=bv[:] if has_bias else None)
                    vt = vp.tile([P, NH, HD + 1], BF16, tag="v")
                    nc.scalar.activation(vt[:, :, 0:HD],
                                         ps.rearrange("p (h d) -> p h d", h=NH),
                                         AF.Copy)
                    nc.vector.memset(vt[:, :, HD:HD + 1], 1.0)
                    vtiles[kb] = vt

                if _phase == "v" and l == n_layers - 1:
                    continue
                # q^T / k^T with RoPE
                qT = qk.tile([P, H // P, T], BF16, tag="qT")
                kT = qk.tile([P, H // P, T], BF16, tag="kT")
                for (dst, w, bias_t, blk0) in (
                    (qT, wq, "bq", qb0),
                    (kT, wk, "bk", kb0),
                ):
                    for oc in range(H // P):
                        for (s0, s1) in _spans(blk0, NB):
                            n = (s1 - s0) * P
                            c0 = s0 * P
                            ps = mm_ps.tile([P, 512], F32, tag="mm", name="mmps")[:, :n]
                            for fc in range(H // P):
                                nc.tensor.matmul(ps, w[:, fc, oc * P:(oc + 1) * P],
                                                 hT[:, fc, c0:c0 + n],
                                                 start=(fc == 0),
                                                 stop=(fc == H // P - 1))
                            q0 = work.tile([P, 512], BF16, tag="q0", name="q0t")[:, :n]
                            if has_bias:
                                bt = bq if bias_t == "bq" else bk
                                nc.scalar.activation(q0, ps, AF.Copy,
                                                     bias=bt[:, oc:oc + 1])
                            else:
                                nc.scalar.activation(q0, ps, AF.Copy)
                            # rope: out = q0 * cs + rot_half(q0) * sn,
                            # rot_half via signed-permutation matmul on PE
                            rp = mm_ps.tile([P, 512], F32, tag="mm", name="rpps")[:, :n]
                            nc.tensor.matmul(rp, rotm[:], q0, start=True, stop=True)
                            t1 = work.tile([P, 512], BF16, tag="t1", name="t1t")[:, :n]
                            nc.vector.tensor_mul(t1, rp, snT[:, c0:c0 + n])
                            t2 = work.tile([P, 512], BF16, tag="t2", name="t2t")[:, :n]
                            nc.vector.tensor_mul(t2, q0, csT[:, c0:c0 + n])
                            nc.vector.tensor_add(dst[:, oc, c0:c0 + n], t1, t2)

                if _phase == "qk" and l == n_layers - 1:
                    continue
                # scores + exp per (kb), then PV/Wo for qb == kb
                estiles = {}
                for kb in range(kb0, NB):
                    qlo, qhi = max(kb, qb0), min(kb + 2, NB)
                    n = (qhi - qlo) * P
                    c0 = qlo * P
                    moff = (qlo - kb) * P
                    for h in range(NH):
                        hp0 = 64 * (h % 2)
                        hc = h // 2
                        sp = s_ps.tile([P, 2 * P], F32, tag="s", name="spt")[:, :n]
                        nc.tensor.matmul(sp,
                                         kT[hp0:hp0 + 64, hc, kb * P:(kb + 1) * P],
                                         qT[hp0:hp0 + 64, hc, c0:c0 + n],
                                         start=True, stop=True)
                        nc.vector.tensor_add(sp, sp, maskT[:, kb, moff:moff + n])
                        est = es.tile([P, 2 * P], BF16, tag=f"es{h}")
                        nc.scalar.activation(est[:, moff:moff + n], sp, AF.Exp,
                                             scale=0.125)
                        estiles[(h, kb)] = est

                    if kb < qb0 or _phase == "scores":
                        continue
                    qb = kb
                    # PV with appended-ones denominator column
                    ops_ = [o_ps.tile([P, 4, HD + 1], F32, tag="o", name=f"opst{_g}") for _g in range(2)]
                    for h in range(NH):
                        sl = ops_[h // 4][:, h % 4, :]
                        nc.tensor.matmul(sl, estiles[(h, qb)][:, 0:P],
                                         vtiles[qb][:, h, :], start=True, stop=False)
                        nc.tensor.matmul(sl, estiles[(h, qb - 1)][:, P:2 * P],
                                         vtiles[qb - 1][:, h, :], start=False, stop=True)
                    if _phase == "pv1":
                        continue
                    den = small.tile([P, NH], F32, tag="den")
                    nc.scalar.activation(den[:, 0:4], ops_[0][:, :, HD], AF.Copy)
                    nc.scalar.activation(den[:, 4:8], ops_[1][:, :, HD], AF.Copy)
                    nc.vector.reciprocal(den[:], den[:])
                    if _phase == "pv2":
                        continue
                    osc = work.tile([P, H], BF16, tag="osc")
                    for g in range(2):
                        nc.vector.tensor_mul(
                            osc.rearrange("p (g2 h d) -> p g2 h d", g2=2, h=4)[:, g],
                            ops_[g][:, :, 0:HD],
                            den[:, g * 4:(g + 1) * 4, None].to_broadcast((P, 4, HD)))
                    if _phase == "pv":
                        continue
                    oT = work.tile([P, H // P, P], BF16, tag="oT")
                    for fc in range(H // P):
                        transpose128(osc[:, fc * P:(fc + 1) * P], oT[:, fc, :])
                    ps = mm_ps.tile([P, 512], F32, tag="mm")
                    mm_group(ps,
                             [(oT[:, fc, :], wo[:, fc, :]) for fc in range(H // P)],
                             bias_row=bo[:] if has_bias else None)
                    nc.vector.tensor_add(x[:, qb, :], ps, x[:, qb, :])

                if _phase == "attn" and l == n_layers - 1:
                    continue
                # ---- MLP ----
                h2T = hTs.tile([P, H // P, T], BF16, tag="hT")
                for qb in range(qb0, NB):
                    hrow = work.tile([P, H], BF16, tag="hrow")
                    layernorm(x[:, qb, :], hrow[:])
                    for fc in range(H // P):
                        transpose128(hrow[:, fc * P:(fc + 1) * P],
                                     h2T[:, fc, qb * P:(qb + 1) * P])

                for (s0, s1) in _spans(qb0, NB):
                    n = (s1 - s0) * P
                    c0 = s0 * P
                    it = itp.tile([P, INTER // P, 512], BF16, tag="iT")
                    for icg in range(2):
                        uw = wts.tile([P, H // P, INTER // 2], BF16, tag="upw")
                        nc.sync.dma_start(
                            out=uw[:],
                            in_=d_upw[l].ap().rearrange("(f p) i -> p f i", p=P)[
                                :, :, icg * (INTER // 2):(icg + 1) * (INTER // 2)])
                        for ic in range(INTER // 2 // P):
                            icx = icg * (INTER // 2 // P) + ic
                            ps = mm_ps.tile([P, 512], F32, tag="mm", name="mmps")[:, :n]
                            for fc in range(H // P):
                                nc.tensor.matmul(ps, uw[:, fc, ic * P:(ic + 1) * P],
                                                 h2T[:, fc, c0:c0 + n],
                                                 start=(fc == 0),
                                                 stop=(fc == H // P - 1))
                            bias = upb[:, icx:icx + 1] if has_bias else 0.0
                            nc.scalar.activation(it[:, icx, :n], ps, AF.Gelu,
                                                 bias=bias)
                    dw = [None, None]
                    for icg in range(2):
                        dw[icg] = wts.tile([P, INTER // 2 // P, H], BF16, tag="dnw",
                                           name=f"dnw{icg}")
                        nc.sync.dma_start(
                            out=dw[icg][:],
                            in_=d_dnw[l].ap().rearrange("(g p) o -> p g o", p=P)[
                                :, icg * (INTER // 2 // P):(icg + 1) * (INTER // 2 // P), :])
                    for qb in range(s0, s1):
                        rel = (qb - s0) * P
                        ps = mm_ps.tile([P, 512], F32, tag="mm")
                        mm_group(ps,
                                 [(it[:, icx, rel:rel + P], dw[icx // 8][:, icx % 8, :])
                                  for icx in range(INTER // P)],
                                 bias_row=dnb[:] if has_bias else None)
                        nc.vector.tensor_add(x[:, qb, :], ps, x[:, qb, :])

            # ---- output: local blocks 4..8 ----
            nc.sync.dma_start(
                out=d_out.ap().rearrange("(b p) h -> p b h", p=P),
                in_=x[:, NB // 2:NB, :])

    nc.finalize()
    return nc


def _rope_tables():
    inv = 1.0 / (BASE ** (np.arange(0, HD, 2, dtype=np.float32) / np.float32(HD)))
    t = np.arange(T, dtype=np.float32)
    f = t[:, None] * inv[None, :]                      # [T, HD/2]
    emb = np.concatenate([f, f], axis=-1)              # [T, HD]
    return np.cos(emb).astype(np.float32), np.sin(emb).astype(np.float32)


def _bf16(x):
    return np.ascontiguousarray(np.asarray(x, np.float32)).astype(ml_dtypes.bfloat16)


def prepare(inputs):
    """Host-side preprocessing: returns (nc, in_maps) for the 8 cores."""
    inp = {k: np.asarray(v) for k, v in inputs.items()}
    spikes = inp["spikes"].astype(np.float32)          # [B, T, C]
    spikes_mask = inp["spikes_mask"].astype(np.int32)  # [B, T]
    ts = inp["spikes_timestamp"].astype(np.int64)      # [B, T]

    # ---- fold LN gains/biases into weights host-side ----
    ln1_g, ln1_b = inp["ln1_g"].astype(np.float32), inp["ln1_b"].astype(np.float32)
    ln2_g, ln2_b = inp["ln2_g"].astype(np.float32), inp["ln2_b"].astype(np.float32)
    Wq, Wk, Wv, Wo = (inp[k].astype(np.float32) for k in ("Wq", "Wk", "Wv", "Wo"))
    upw, dnw = inp["up_w"].astype(np.float32), inp["down_w"].astype(np.float32)
    bq = inp["bq"].astype(np.float32) + np.einsum("lh,lho->lo", ln1_b, Wq)
    bk = inp["bk"].astype(np.float32) + np.einsum("lh,lho->lo", ln1_b, Wk)
    bv = inp["bv"].astype(np.float32) + np.einsum("lh,lho->lo", ln1_b, Wv)
    bo = inp["bo"].astype(np.float32)
    upb = inp["up_b"].astype(np.float32) + np.einsum("lh,lhi->li", ln2_b, upw)
    dnb = inp["down_b"].astype(np.float32)
    wq_eff = ln1_g[:, :, None] * Wq
    wk_eff = ln1_g[:, :, None] * Wk
    wv_eff = ln1_g[:, :, None] * Wv
    upw_eff = ln2_g[:, :, None] * upw

    has_bias = bool(
        np.abs(inp["embed_b"]).max() > 0 or np.abs(inp["proj_b"]).max() > 0
        or max(np.abs(a).max() for a in (bq, bk, bv, bo, upb, dnb)) > 0)

    key = has_bias
    if key not in _PROG_CACHE:
        _PROG_CACHE[key] = _build_program(has_bias)
    nc = _PROG_CACHE[key]

    # ---- shared weight arrays ----
    shared = {
        "embw": _bf16(inp["embed_w"]),
        "projw": _bf16(inp["proj_w"]),
    }
    for l in range(L):
        shared[f"wq{l}"] = _bf16(wq_eff[l])
        shared[f"wk{l}"] = _bf16(wk_eff[l])
        shared[f"wv{l}"] = _bf16(wv_eff[l])
        shared[f"wo{l}"] = _bf16(Wo[l])
        shared[f"upw{l}"] = _bf16(upw_eff[l])
        shared[f"dnw{l}"] = _bf16(dnw[l])
    if has_bias:
        shared["embb"] = inp["embed_b"].astype(np.float32)
        shared["projb"] = _bf16(inp["proj_b"]).reshape(1, H)
        for l in range(L):
            shared[f"bq{l}"] = bq[l]
            shared[f"bk{l}"] = bk[l]
            shared[f"bv{l}"] = _bf16(bv[l]).reshape(1, H)
            shared[f"bo{l}"] = _bf16(bo[l]).reshape(1, H)
            shared[f"upb{l}"] = upb[l]
            shared[f"dnb{l}"] = _bf16(dnb[l]).reshape(1, H)

    cos_t, sin_t = _rope_tables()   # [T, HD]

    # signed permutation for rotate-half: out[m] = sign(m) * q[partner(m)]
    # (as matmul rotm.T @ q: rotm[partner(m), m] = sign(m))
    rotm_np = np.zeros((P, P), np.float32)
    for m in range(P):
        d = m % HD
        partner = m + HD // 2 if d < HD // 2 else m - HD // 2
        rotm_np[partner, m] = -1.0 if d < HD // 2 else 1.0
    rotm_np = _bf16(rotm_np)

    in_maps = []
    for b in range(B):
        for h in range(2):
            g0 = h * (T // 2)       # global row of local row 512
            # local row r -> global row r - 512 + g0
            gl = np.arange(T) - (T // 2) + g0
            valid = gl >= 0
            glc = np.clip(gl, 0, T - 1)

            spT_local = np.zeros((C, T), np.float32)
            spT_local[:, valid] = spikes[b, glc[valid], :].T

            ts_local = np.where(valid, ts[b, glc], 0)
            cs_l = cos_t[ts_local]          # [T(local), HD]
            sn_l = sin_t[ts_local]
            # feature-major rope tables [128, T]: partition p -> d = p % 64,
            # sign of sn negative for d < 32 (rot-half sign fold)
            d_of_p = np.arange(P) % HD
            csT_l = cs_l[:, d_of_p].T.astype(np.float32)            # [128, T]
            snT_l = sn_l[:, d_of_p].T.astype(np.float32)

            # additive mask bias tiles [kb, kc, qcol(2 blocks)]
            km = np.zeros((NB, P, 2 * P), np.float32)
            kc = np.arange(P)
            for kb in range(NB):
                lk = kb * P + kc                      # local key row
                gk = lk - (T // 2) + g0
                for dq in range(2):
                    qb = kb + dq
                    if qb >= NB:
                        continue
                    lq = qb * P + np.arange(P)
                    gq = lq - (T // 2) + g0
                    allowed = ((gk[:, None] >= 0)
                               & (gk[:, None] <= gq[None, :] + CF)
                               & (gk[:, None] >= gq[None, :] - CB))
                    allowed &= (spikes_mask[b, np.clip(gk, 0, T - 1)] > 0)[:, None]
                    bias = np.where(allowed, 0.0, NEG)
                    # pad queries (gq < 0) attend everything (keeps denom > 0)
                    bias[:, gq < 0] = 0.0
                    km[kb, :, dq * P:(dq + 1) * P] = bias

            in_maps.append(dict(
                shared,
                rotm=rotm_np,
                spikesT=_bf16(spT_local),
                csT=csT_l,
                snT=snT_l,
                maskT=km,
            ))

    return nc, in_maps


def kernel(**inputs):
    nc, in_maps = prepare(inputs)
    r = run_bass_kernel_spmd(nc, in_maps, core_ids=list(range(N_CORES)))
    out = np.empty((B, T, H), np.float32)
    for b in range(B):
        for h in range(2):
            out[b, h * (T // 2):(h + 1) * (T // 2), :] = r.results[b * 2 + h]["out"]
    return out



# revision 3
# speedup vs baseline: 14.8203x; 14.8203x over previous
"""Trainium2 Bass kernel for nn_NeuralEncoder (sparse banded attention encoder).

Sharding: 8 cores = (batch b in 0..3) x (sequence half h in 0..1), uniform SPMD
program over a 1024-row local window per core: h=0 cores get 512 zero-pad rows +
rows 0..511, h=1 cores get rows 0..1023. Each layer shrinks the active window by
128 rows at the front (the CB=128 sliding-window halo); every core emits local
rows 512..1023 as its 512 output rows.

Host<->device transfer is the bottleneck (axon tunnel ~50MB/s, serialized), so
all inputs ship as ONE bf16 wire tensor to core 0 only; cores 1-7 receive
device-created zeros. On device an AllReduce(add) broadcasts the shared weight
blob and a ReduceScatter(add) hands each core its private window data
(spikes/rope tables/mask). Output returns in bf16.

Numerics: bf16 matmuls with fp32 PSUM accumulation; LayerNorm, softmax and the
residual stream in fp32. LN gains are folded into the following weight matrices
host-side; the band/padding/spikes_mask is a host-precomputed additive bias
applied to attention scores pre-exp.
"""

import os
import sys

for _p in ("/opt/trn_rl_repo", "/root/.axon_site/_ro/trn_rl_repo"):
    if _p not in sys.path and os.path.isdir(_p):
        sys.path.append(_p)

import numpy as np
import ml_dtypes
import jax
import jax.numpy as jnp
from jax.sharding import Mesh, PartitionSpec, NamedSharding, SingleDeviceSharding
try:
    from jax.experimental.shard_map import shard_map
except ImportError:
    from jax import shard_map

from concourse import bacc
import concourse.tile as tile
from concourse import mybir
from concourse import bass2jax
from concourse.masks import make_identity

# dims
B, T, C, D, H, NH, HD, INTER, L = 4, 1024, 256, 256, 512, 8, 64, 2048, 4
CF, CB, BASE = 0, 128, 10000.0
P = 128
NB = T // P          # 8 local row blocks
N_CORES = 8
NEG = np.float32(-1e30)
F32 = mybir.dt.float32
BF16 = mybir.dt.bfloat16
AF = mybir.ActivationFunctionType

_PROG_CACHE = {}
_EXEC_CACHE = {}


# ---------------------------------------------------------------------------
# wire layout (bf16 elems). Blob = broadcast (shared) region; PC = per-core.
# ---------------------------------------------------------------------------

def _blob_layout(has_bias):
    regions = [("rotm", P * P), ("embw", P * 512), ("projw", P * 1024)]
    for l in range(L):
        for nm in ("wq", "wk", "wv", "wo"):
            regions.append((f"{nm}{l}", P * 2048))
        regions.append((f"upw{l}", P * 8192))
        regions.append((f"dnw{l}", P * 8192))
    if has_bias:
        regions.append(("embb", P * 2))
        regions.append(("projb", H))
        for l in range(L):
            regions.append((f"bq{l}", P * 4))
            regions.append((f"bk{l}", P * 4))
            regions.append((f"bv{l}", H))
            regions.append((f"bo{l}", H))
            regions.append((f"upb{l}", P * 16))
            regions.append((f"dnb{l}", H))
    off, out = 0, {}
    for name, n in regions:
        out[name] = (off, n)
        off += n
    return out, off


# per-core region: offsets within each core's PCW-elem chunk
_PC_SPT = 0                    # [128, 2, 1024]
_PC_CST = P * 2048             # [64, 1024]
_PC_SNT = _PC_CST + 64 * 1024  # [64, 1024]
_PC_MSK = _PC_SNT + 64 * 1024  # [128, 8, 256]
PCW = _PC_MSK + P * 8 * 256


def _spans(start_block, end_block, max_blocks=4):
    """Split block range [start_block, end_block) into runs of <= max_blocks."""
    out = []
    b = start_block
    while b < end_block:
        e = min(b + max_blocks, end_block)
        out.append((b, e))
        b = e
    return out


def _build_program(has_bias):
    blob_off, blob_elems = _blob_layout(has_bias)
    nw = blob_elems + N_CORES * PCW

    nc = bacc.Bacc("TRN2", target_bir_lowering=False, debug=False,
                   num_devices=N_CORES)

    d_wire = nc.dram_tensor("wire", [nw], BF16, kind="ExternalInput")
    d_blob_in = nc.dram_tensor("blob_in", [blob_elems], BF16)
    d_blob = nc.dram_tensor("blob", [blob_elems], BF16, addr_space="Shared")
    d_pc_in = nc.dram_tensor("pc_in", [N_CORES * PCW], BF16)
    d_pc = nc.dram_tensor("pc", [PCW], BF16)
    d_out = nc.dram_tensor("out", [T // 2, H], BF16, kind="ExternalOutput")

    def bvw(name, pat, **dims):
        off, n = blob_off[name]
        ap = d_blob.ap()[off:off + n]
        return ap.rearrange(pat, **dims) if pat else ap

    def pcv(off, n, pat, **dims):
        ap = d_pc.ap()[off:off + n]
        return ap.rearrange(pat, **dims) if pat else ap

    with tile.TileContext(nc) as tc:
        with (
            tc.tile_pool(name="consts", bufs=1) as consts,
            tc.tile_pool(name="wts", bufs=2) as wts,
            tc.tile_pool(name="work", bufs=2) as work,
            tc.tile_pool(name="small", bufs=6) as small,
            tc.tile_pool(name="hTs", bufs=2) as hTs,
            tc.tile_pool(name="qk", bufs=1) as qk,
            tc.tile_pool(name="vp", bufs=9) as vp,
            tc.tile_pool(name="es", bufs=3) as es,
            tc.tile_pool(name="itp", bufs=1) as itp,
            tc.tile_pool(name="mm_ps", bufs=3, space="PSUM") as mm_ps,
            tc.tile_pool(name="s_ps", bufs=2, space="PSUM") as s_ps,
            tc.tile_pool(name="o_ps", bufs=2, space="PSUM") as o_ps,
            tc.tile_pool(name="t_ps", bufs=1, space="PSUM") as t_ps,
        ):
            # ---- distribute the wire: broadcast blob, scatter per-core ----
            nc.sync.dma_start(out=d_blob_in.ap(), in_=d_wire.ap()[0:blob_elems])
            nc.gpsimd.collective_compute(
                "AllReduce", mybir.AluOpType.add,
                replica_groups=[list(range(N_CORES))],
                ins=[d_blob_in.ap()], outs=[d_blob.ap()])
            nc.sync.dma_start(out=d_pc_in.ap(), in_=d_wire.ap()[blob_elems:nw])
            nc.gpsimd.collective_compute(
                "ReduceScatter", mybir.AluOpType.add,
                replica_groups=[list(range(N_CORES))],
                ins=[d_pc_in.ap()], outs=[d_pc.ap()])

            # ---- constants ----
            ident = consts.tile([P, P], BF16, tag="ident")
            make_identity(nc, ident[:])
            eps = consts.tile([P, 1], F32, tag="eps")
            nc.vector.memset(eps[:], 1e-5)
            csT = consts.tile([P, T], BF16, tag="csT")
            nc.sync.dma_start(out=csT[0:64, :],
                              in_=pcv(_PC_CST, 64 * T, "(p r) -> p r", p=64))
            nc.sync.dma_start(out=csT[64:128, :],
                              in_=pcv(_PC_CST, 64 * T, "(p r) -> p r", p=64))
            snT = consts.tile([P, T], BF16, tag="snT")
            nc.sync.dma_start(out=snT[0:64, :],
                              in_=pcv(_PC_SNT, 64 * T, "(p r) -> p r", p=64))
            nc.sync.dma_start(out=snT[64:128, :],
                              in_=pcv(_PC_SNT, 64 * T, "(p r) -> p r", p=64))
            maskT = consts.tile([P, NB, 2 * P], BF16, tag="maskT")
            nc.sync.dma_start(out=maskT[:],
                              in_=pcv(_PC_MSK, P * NB * 2 * P,
                                      "(p k q) -> p k q", p=P, k=NB))
            spT = consts.tile([P, C // P, T], BF16, tag="spT")
            nc.sync.dma_start(out=spT[:],
                              in_=pcv(_PC_SPT, P * 2 * T,
                                      "(p c r) -> p c r", p=P, c=C // P))
            rotm = consts.tile([P, P], BF16, tag="rotm")
            nc.sync.dma_start(out=rotm[:], in_=bvw("rotm", "(p m) -> p m", p=P))
            embw = consts.tile([P, C // P, D], BF16, tag="embw")
            nc.sync.dma_start(out=embw[:],
                              in_=bvw("embw", "(p c d) -> p c d", p=P, c=C // P))
            projw = consts.tile([P, D // P, H], BF16, tag="projw")
            nc.sync.dma_start(out=projw[:],
                              in_=bvw("projw", "(p c h) -> p c h", p=P, c=D // P))
            if has_bias:
                embb_b = consts.tile([P, D // P], BF16, tag="embb_b")
                nc.sync.dma_start(out=embb_b[:],
                                  in_=bvw("embb", "(p c) -> p c", p=P))
                embb = consts.tile([P, D // P], F32, tag="embb")
                nc.scalar.activation(embb[:], embb_b[:], AF.Copy)
                projb = consts.tile([1, H], BF16, tag="projb")
                nc.sync.dma_start(out=projb[:], in_=bvw("projb", "(a h) -> a h", a=1))
                ones_r = consts.tile([1, P], BF16, tag="ones_r")
                nc.vector.memset(ones_r[:], 1.0)

            x = consts.tile([P, NB, H], F32, tag="x")
            gT = consts.tile([P, D // P, T], BF16, tag="gT")

            def mm_group(ps, pairs, bias_row=None):
                """Accumulate lhsT.T @ rhs pairs into ps; optional bias row
                (psum += ones^T @ bias_row) closes the group."""
                for i, (a, bb) in enumerate(pairs):
                    last = (i == len(pairs) - 1) and bias_row is None
                    nc.tensor.matmul(ps, a, bb, start=(i == 0), stop=last)
                if bias_row is not None:
                    nc.tensor.matmul(ps, ones_r[:], bias_row,
                                     start=False, stop=True)

            # ---- embedding: gT = gelu(spikes @ embed_w)^T, x = gT^T @ proj_w ----
            for oc in range(D // P):
                for (s0, s1) in _spans(0, NB):
                    n = (s1 - s0) * P
                    ps = mm_ps.tile([P, 512], F32, tag="mm", name="mmps")[:, :n]
                    for fc in range(C // P):
                        nc.tensor.matmul(ps, embw[:, fc, oc * P:(oc + 1) * P],
                                         spT[:, fc, s0 * P:s0 * P + n],
                                         start=(fc == 0), stop=(fc == C // P - 1))
                    bias = embb[:, oc:oc + 1] if has_bias else 0.0
                    nc.scalar.activation(gT[:, oc, s0 * P:s0 * P + n], ps, AF.Gelu,
                                         bias=bias)
            for rb in range(NB):
                ps = mm_ps.tile([P, 512], F32, tag="mm")
                mm_group(ps,
                         [(gT[:, fc, rb * P:(rb + 1) * P], projw[:, fc, :])
                          for fc in range(D // P)],
                         bias_row=projb[:] if has_bias else None)
                nc.scalar.activation(x[:, rb, :], ps, AF.Copy)

            # ---- layers ----
            _trunc = os.environ.get("KTRUNC", "")
            n_layers = L
            if _trunc.startswith("L"):
                n_layers = int(_trunc[1:].split(":")[0])
            _phase = _trunc.split(":")[1] if ":" in _trunc else "all"
            for l in range(n_layers):
                kb0, qb0 = l, l + 1

                wq = wts.tile([P, H // P, H], BF16, tag="wq")
                nc.sync.dma_start(out=wq[:],
                                  in_=bvw(f"wq{l}", "(p f o) -> p f o", p=P, f=H // P))
                wk = wts.tile([P, H // P, H], BF16, tag="wk")
                nc.sync.dma_start(out=wk[:],
                                  in_=bvw(f"wk{l}", "(p f o) -> p f o", p=P, f=H // P))
                wv = wts.tile([P, H // P, H], BF16, tag="wv")
                nc.sync.dma_start(out=wv[:],
                                  in_=bvw(f"wv{l}", "(p f o) -> p f o", p=P, f=H // P))
                wo = wts.tile([P, H // P, H], BF16, tag="wo")
                nc.sync.dma_start(out=wo[:],
                                  in_=bvw(f"wo{l}", "(p f o) -> p f o", p=P, f=H // P))
                if has_bias:
                    bq_b = wts.tile([P, H // P], BF16, tag="bq_b")
                    nc.sync.dma_start(out=bq_b[:],
                                      in_=bvw(f"bq{l}", "(p c) -> p c", p=P))
                    bq = wts.tile([P, H // P], F32, tag="bq")
                    nc.scalar.activation(bq[:], bq_b[:], AF.Copy)
                    bk_b = wts.tile([P, H // P], BF16, tag="bk_b")
                    nc.sync.dma_start(out=bk_b[:],
                                      in_=bvw(f"bk{l}", "(p c) -> p c", p=P))
                    bk = wts.tile([P, H // P], F32, tag="bk")
                    nc.scalar.activation(bk[:], bk_b[:], AF.Copy)
                    bv = wts.tile([1, H], BF16, tag="bv")
                    nc.sync.dma_start(out=bv[:], in_=bvw(f"bv{l}", "(a h) -> a h", a=1))
                    bo = wts.tile([1, H], BF16, tag="bo")
                    nc.sync.dma_start(out=bo[:], in_=bvw(f"bo{l}", "(a h) -> a h", a=1))
                    dnb = wts.tile([1, H], BF16, tag="dnb")
                    nc.sync.dma_start(out=dnb[:],
                                      in_=bvw(f"dnb{l}", "(a h) -> a h", a=1))
                    upb_b = wts.tile([P, INTER // P], BF16, tag="upb_b")
                    nc.sync.dma_start(out=upb_b[:],
                                      in_=bvw(f"upb{l}", "(p c) -> p c", p=P))
                    upb = wts.tile([P, INTER // P], F32, tag="upb")
                    nc.scalar.activation(upb[:], upb_b[:], AF.Copy)

                def layernorm(src_ap, dst_bf16_ap):
                    stats = small.tile([P, 6], F32, tag="stats")
                    nc.vector.bn_stats(stats[:], src_ap)
                    mv = small.tile([P, 2], F32, tag="mv")
                    nc.vector.bn_aggr(mv[:], stats[:])
                    rstd = small.tile([P, 1], F32, tag="rstd")
                    nc.scalar.activation(rstd[:], mv[:, 1:2], AF.Sqrt, bias=eps[:])
                    nc.vector.reciprocal(rstd[:], rstd[:])
                    nc.vector.tensor_scalar(dst_bf16_ap, src_ap,
                                            mv[:, 0:1], rstd[:],
                                            mybir.AluOpType.subtract,
                                            mybir.AluOpType.mult)

                def transpose128(src_bf16_ap, dst_bf16_ap):
                    # src [128, 128] -> dst [128, 128] via PE transpose
                    tp = t_ps.tile([P, P], BF16, tag="tp")
                    nc.tensor.transpose(tp[:], src_bf16_ap, ident[:])
                    nc.scalar.activation(dst_bf16_ap, tp[:], AF.Copy)

                # LN1 + h^T + v for key range
                hT = hTs.tile([P, H // P, T], BF16, tag="hT")
                vtiles = {}
                for kb in range(kb0, NB):
                    hrow = work.tile([P, H], BF16, tag="hrow")
                    layernorm(x[:, kb, :], hrow[:])
                    for fc in range(H // P):
                        transpose128(hrow[:, fc * P:(fc + 1) * P],
                                     hT[:, fc, kb * P:(kb + 1) * P])
                    ps = mm_ps.tile([P, 512], F32, tag="mm")
                    mm_group(ps,
                             [(hT[:, fc, kb * P:(kb + 1) * P], wv[:, fc, :])
                              for fc in range(H // P)],
                             bias_row=bv[:] if has_bias else None)
                    vt = vp.tile([P, NH, HD + 1], BF16, tag="v")
                    nc.scalar.activation(vt[:, :, 0:HD],
                                         ps.rearrange("p (h d) -> p h d", h=NH),
                                         AF.Copy)
                    nc.vector.memset(vt[:, :, HD:HD + 1], 1.0)
                    vtiles[kb] = vt

                if _phase == "v" and l == n_layers - 1:
                    continue
                # q^T / k^T with RoPE
                qT = qk.tile([P, H // P, T], BF16, tag="qT")
                kT = qk.tile([P, H // P, T], BF16, tag="kT")
                for (dst, w, bias_t, blk0) in (
                    (qT, wq, "bq", qb0),
                    (kT, wk, "bk", kb0),
                ):
                    for oc in range(H // P):
                        for (s0, s1) in _spans(blk0, NB):
                            n = (s1 - s0) * P
                            c0 = s0 * P
                            ps = mm_ps.tile([P, 512], F32, tag="mm", name="mmps")[:, :n]
                            for fc in range(H // P):
                                nc.tensor.matmul(ps, w[:, fc, oc * P:(oc + 1) * P],
                                                 hT[:, fc, c0:c0 + n],
                                                 start=(fc == 0),
                                                 stop=(fc == H // P - 1))
                            q0 = work.tile([P, 512], BF16, tag="q0", name="q0t")[:, :n]
                            if has_bias:
                                bt = bq if bias_t == "bq" else bk
                                nc.scalar.activation(q0, ps, AF.Copy,
                                                     bias=bt[:, oc:oc + 1])
                            else:
                                nc.scalar.activation(q0, ps, AF.Copy)
                            # rope: out = q0 * cs + rot_half(q0) * sn,
                            # rot_half via signed-permutation matmul on PE
                            rp = mm_ps.tile([P, 512], F32, tag="mm", name="rpps")[:, :n]
                            nc.tensor.matmul(rp, rotm[:], q0, start=True, stop=True)
                            t1 = work.tile([P, 512], BF16, tag="t1", name="t1t")[:, :n]
                            nc.vector.tensor_mul(t1, rp, snT[:, c0:c0 + n])
                            t2 = work.tile([P, 512], BF16, tag="t2", name="t2t")[:, :n]
                            nc.vector.tensor_mul(t2, q0, csT[:, c0:c0 + n])
                            nc.vector.tensor_add(dst[:, oc, c0:c0 + n], t1, t2)

                if _phase == "qk" and l == n_layers - 1:
                    continue
                # scores + exp per (kb), then PV/Wo for qb == kb
                estiles = {}
                for kb in range(kb0, NB):
                    qlo, qhi = max(kb, qb0), min(kb + 2, NB)
                    n = (qhi - qlo) * P
                    c0 = qlo * P
                    moff = (qlo - kb) * P
                    for h in range(NH):
                        hp0 = 64 * (h % 2)
                        hc = h // 2
                        sp = s_ps.tile([P, 2 * P], F32, tag="s", name="spt")[:, :n]
                        nc.tensor.matmul(sp,
                                         kT[hp0:hp0 + 64, hc, kb * P:(kb + 1) * P],
                                         qT[hp0:hp0 + 64, hc, c0:c0 + n],
                                         start=True, stop=True)
                        nc.vector.tensor_add(sp, sp, maskT[:, kb, moff:moff + n])
                        est = es.tile([P, 2 * P], BF16, tag=f"es{h}")
                        nc.scalar.activation(est[:, moff:moff + n], sp, AF.Exp,
                                             scale=0.125)
                        estiles[(h, kb)] = est

                    if kb < qb0 or _phase == "scores":
                        continue
                    qb = kb
                    # PV with appended-ones denominator column
                    ops_ = [o_ps.tile([P, 4, HD + 1], F32, tag="o", name=f"opst{_g}") for _g in range(2)]
                    for h in range(NH):
                        sl = ops_[h // 4][:, h % 4, :]
                        nc.tensor.matmul(sl, estiles[(h, qb)][:, 0:P],
                                         vtiles[qb][:, h, :], start=True, stop=False)
                        nc.tensor.matmul(sl, estiles[(h, qb - 1)][:, P:2 * P],
                                         vtiles[qb - 1][:, h, :], start=False, stop=True)
                    if _phase == "pv1":
                        continue
                    den = small.tile([P, NH], F32, tag="den")
                    nc.scalar.activation(den[:, 0:4], ops_[0][:, :, HD], AF.Copy)
                    nc.scalar.activation(den[:, 4:8], ops_[1][:, :, HD], AF.Copy)
                    nc.vector.reciprocal(den[:], den[:])
                    if _phase == "pv2":
                        continue
                    osc = work.tile([P, H], BF16, tag="osc")
                    for g in range(2):
                        nc.vector.tensor_mul(
                            osc.rearrange("p (g2 h d) -> p g2 h d", g2=2, h=4)[:, g],
                            ops_[g][:, :, 0:HD],
                            den[:, g * 4:(g + 1) * 4, None].to_broadcast((P, 4, HD)))
                    if _phase == "pv":
                        continue
                    oT = work.tile([P, H // P, P], BF16, tag="oT")
                    for fc in range(H // P):
                        transpose128(osc[:, fc * P:(fc + 1) * P], oT[:, fc, :])
                    ps = mm_ps.tile([P, 512], F32, tag="mm")
                    mm_group(ps,
                             [(oT[:, fc, :], wo[:, fc, :]) for fc in range(H // P)],
                             bias_row=bo[:] if has_bias else None)
                    nc.vector.tensor_add(x[:, qb, :], ps, x[:, qb, :])

                if _phase == "attn" and l == n_layers - 1:
                    continue
                # ---- MLP ----
                h2T = hTs.tile([P, H // P, T], BF16, tag="hT")
                for qb in range(qb0, NB):
                    hrow = work.tile([P, H], BF16, tag="hrow")
                    layernorm(x[:, qb, :], hrow[:])
                    for fc in range(H // P):
                        transpose128(hrow[:, fc * P:(fc + 1) * P],
                                     h2T[:, fc, qb * P:(qb + 1) * P])

                for (s0, s1) in _spans(qb0, NB):
                    n = (s1 - s0) * P
                    c0 = s0 * P
                    it = itp.tile([P, INTER // P, 512], BF16, tag="iT")
                    for icg in range(2):
                        uw = wts.tile([P, H // P, INTER // 2], BF16, tag="upw")
                        nc.sync.dma_start(
                            out=uw[:],
                            in_=bvw(f"upw{l}", "(p f i) -> p f i", p=P, f=H // P)[
                                :, :, icg * (INTER // 2):(icg + 1) * (INTER // 2)])
                        for ic in range(INTER // 2 // P):
                            icx = icg * (INTER // 2 // P) + ic
                            ps = mm_ps.tile([P, 512], F32, tag="mm", name="mmps")[:, :n]
                            for fc in range(H // P):
                                nc.tensor.matmul(ps, uw[:, fc, ic * P:(ic + 1) * P],
                                                 h2T[:, fc, c0:c0 + n],
                                                 start=(fc == 0),
                                                 stop=(fc == H // P - 1))
                            bias = upb[:, icx:icx + 1] if has_bias else 0.0
                            nc.scalar.activation(it[:, icx, :n], ps, AF.Gelu,
                                                 bias=bias)
                    dw = [None, None]
                    for icg in range(2):
                        dw[icg] = wts.tile([P, INTER // 2 // P, H], BF16, tag="dnw",
                                           name=f"dnw{icg}")
                        nc.sync.dma_start(
                            out=dw[icg][:],
                            in_=bvw(f"dnw{l}", "(p g o) -> p g o", p=P, g=INTER // P)[
                                :, icg * (INTER // 2 // P):(icg + 1) * (INTER // 2 // P), :])
                    for qb in range(s0, s1):
                        rel = (qb - s0) * P
                        ps = mm_ps.tile([P, 512], F32, tag="mm")
                        mm_group(ps,
                                 [(it[:, icx, rel:rel + P], dw[icx // 8][:, icx % 8, :])
                                  for icx in range(INTER // P)],
                                 bias_row=dnb[:] if has_bias else None)
                        nc.vector.tensor_add(x[:, qb, :], ps, x[:, qb, :])

            # ---- output: local blocks 4..8, converted to bf16 ----
            xout = consts.tile([P, NB // 2, H], BF16, tag="xout")
            nc.scalar.activation(xout[:], x[:, NB // 2:NB, :], AF.Copy)
            nc.sync.dma_start(
                out=d_out.ap().rearrange("(b p) h -> p b h", p=P),
                in_=xout[:])

    nc.finalize()
    return nc


def _rope_tables():
    inv = 1.0 / (BASE ** (np.arange(0, HD, 2, dtype=np.float32) / np.float32(HD)))
    t = np.arange(T, dtype=np.float32)
    f = t[:, None] * inv[None, :]                      # [T, HD/2]
    emb = np.concatenate([f, f], axis=-1)              # [T, HD]
    return np.cos(emb).astype(np.float32), np.sin(emb).astype(np.float32)


def _bf16(x):
    return np.ascontiguousarray(np.asarray(x, np.float32)).astype(ml_dtypes.bfloat16)


def _pmajor(w, p_groups):
    """[G*128, X] row-major -> [128, G, X] p-major, raveled (bf16)."""
    w = np.asarray(w)
    g = w.shape[0] // P
    return _bf16(w.reshape(g, P, -1).transpose(1, 0, 2)).ravel()


def prepare(inputs):
    """Host-side preprocessing: returns (nc, wire0) — wire0 ships to core 0."""
    inp = {k: np.asarray(v) for k, v in inputs.items()}
    spikes = inp["spikes"].astype(np.float32)          # [B, T, C]
    spikes_mask = inp["spikes_mask"].astype(np.int32)  # [B, T]
    ts = inp["spikes_timestamp"].astype(np.int64)      # [B, T]

    # ---- fold LN gains/biases into weights host-side ----
    ln1_g, ln1_b = inp["ln1_g"].astype(np.float32), inp["ln1_b"].astype(np.float32)
    ln2_g, ln2_b = inp["ln2_g"].astype(np.float32), inp["ln2_b"].astype(np.float32)
    Wq, Wk, Wv, Wo = (inp[k].astype(np.float32) for k in ("Wq", "Wk", "Wv", "Wo"))
    upw, dnw = inp["up_w"].astype(np.float32), inp["down_w"].astype(np.float32)
    bq = inp["bq"].astype(np.float32) + np.einsum("lh,lho->lo", ln1_b, Wq)
    bk = inp["bk"].astype(np.float32) + np.einsum("lh,lho->lo", ln1_b, Wk)
    bv = inp["bv"].astype(np.float32) + np.einsum("lh,lho->lo", ln1_b, Wv)
    bo = inp["bo"].astype(np.float32)
    upb = inp["up_b"].astype(np.float32) + np.einsum("lh,lhi->li", ln2_b, upw)
    dnb = inp["down_b"].astype(np.float32)
    wq_eff = ln1_g[:, :, None] * Wq
    wk_eff = ln1_g[:, :, None] * Wk
    wv_eff = ln1_g[:, :, None] * Wv
    upw_eff = ln2_g[:, :, None] * upw

    has_bias = bool(
        np.abs(inp["embed_b"]).max() > 0 or np.abs(inp["proj_b"]).max() > 0
        or max(np.abs(a).max() for a in (bq, bk, bv, bo, upb, dnb)) > 0)

    key = has_bias
    if key not in _PROG_CACHE:
        _PROG_CACHE[key] = _build_program(has_bias)
    nc = _PROG_CACHE[key]

    blob_off, blob_elems = _blob_layout(has_bias)

    # signed permutation for rotate-half: out[m] = sign(m) * q[partner(m)]
    # (as matmul rotm.T @ q: rotm[partner(m), m] = sign(m))
    rotm_np = np.zeros((P, P), np.float32)
    for m in range(P):
        d = m % HD
        partner = m + HD // 2 if d < HD // 2 else m - HD // 2
        rotm_np[partner, m] = -1.0 if d < HD // 2 else 1.0

    blob = np.zeros(blob_elems, ml_dtypes.bfloat16)

    def put(name, arr_flat):
        off, n = blob_off[name]
        assert arr_flat.size == n, (name, arr_flat.size, n)
        blob[off:off + n] = arr_flat

    put("rotm", _bf16(rotm_np).ravel())
    put("embw", _pmajor(inp["embed_w"], 2))
    put("projw", _pmajor(inp["proj_w"], 2))
    for l in range(L):
        put(f"wq{l}", _pmajor(wq_eff[l], 4))
        put(f"wk{l}", _pmajor(wk_eff[l], 4))
        put(f"wv{l}", _pmajor(wv_eff[l], 4))
        put(f"wo{l}", _pmajor(Wo[l], 4))
        put(f"upw{l}", _pmajor(upw_eff[l], 4))
        put(f"dnw{l}", _pmajor(dnw[l], 16))
    if has_bias:
        put("embb", _bf16(inp["embed_b"].reshape(2, P).T).ravel())
        put("projb", _bf16(inp["proj_b"]).ravel())
        for l in range(L):
            put(f"bq{l}", _bf16(bq[l].reshape(4, P).T).ravel())
            put(f"bk{l}", _bf16(bk[l].reshape(4, P).T).ravel())
            put(f"bv{l}", _bf16(bv[l]).ravel())
            put(f"bo{l}", _bf16(bo[l]).ravel())
            put(f"upb{l}", _bf16(upb[l].reshape(16, P).T).ravel())
            put(f"dnb{l}", _bf16(dnb[l]).ravel())

    cos_t, sin_t = _rope_tables()   # [T, HD]

    pcs = []
    for b in range(B):
        for h in range(2):
            g0 = h * (T // 2)       # global row of local row 512
            # local row r -> global row r - 512 + g0
            gl = np.arange(T) - (T // 2) + g0
            valid = gl >= 0
            glc = np.clip(gl, 0, T - 1)

            spT_local = np.zeros((C, T), np.float32)
            spT_local[:, valid] = spikes[b, glc[valid], :].T

            ts_local = np.where(valid, ts[b, glc], 0)
            cs_l = cos_t[ts_local]          # [T(local), HD]
            sn_l = sin_t[ts_local]
            # feature-major rope tables [64, T]: partition p -> d = p
            csT_l = cs_l[:, 0:HD].T.astype(np.float32)            # [64, T]
            snT_l = sn_l[:, 0:HD].T.astype(np.float32)

            # additive mask bias tiles [kb, kc, qcol(2 blocks)]
            km = np.zeros((NB, P, 2 * P), np.float32)
            kc = np.arange(P)
            for kb in range(NB):
                lk = kb * P + kc                      # local key row
                gk = lk - (T // 2) + g0
                for dq in range(2):
                    qb = kb + dq
                    if qb >= NB:
                        continue
                    lq = qb * P + np.arange(P)
                    gq = lq - (T // 2) + g0
                    allowed = ((gk[:, None] >= 0)
                               & (gk[:, None] <= gq[None, :] + CF)
                               & (gk[:, None] >= gq[None, :] - CB))
                    allowed &= (spikes_mask[b, np.clip(gk, 0, T - 1)] > 0)[:, None]
                    bias = np.where(allowed, 0.0, NEG)
                    # pad queries (gq < 0) attend everything (keeps denom > 0)
                    bias[:, gq < 0] = 0.0
                    km[kb, :, dq * P:(dq + 1) * P] = bias

            pc = np.empty(PCW, ml_dtypes.bfloat16)
            pc[_PC_SPT:_PC_SPT + P * 2 * T] = _pmajor(spT_local, 2)
            pc[_PC_CST:_PC_CST + 64 * T] = _bf16(csT_l).ravel()
            pc[_PC_SNT:_PC_SNT + 64 * T] = _bf16(snT_l).ravel()
            pc[_PC_MSK:_PC_MSK + P * NB * 2 * P] = _bf16(
                km.transpose(1, 0, 2)).ravel()
            pcs.append(pc)

    wire0 = np.concatenate([blob] + pcs)
    return nc, wire0


# ---------------------------------------------------------------------------
# cached-jit runner: wire ships to core 0 only; cores 1..7 get device zeros
# ---------------------------------------------------------------------------

def _get_exec(nc):
    key = id(nc)
    if key in _EXEC_CACHE:
        return _EXEC_CACHE[key]
    bass2jax.install_neuronx_cc_hook()
    partition_name = nc.partition_id_tensor.name if nc.partition_id_tensor else None
    in_names, out_names, out_avals, zero_shapes = [], [], [], []
    for alloc in nc.m.functions[0].allocations:
        if not isinstance(alloc, mybir.MemoryLocationSet):
            continue
        name = alloc.memorylocations[0].name
        if alloc.kind == "ExternalInput":
            if name != partition_name:
                in_names.append(name)
        elif alloc.kind == "ExternalOutput":
            shape = tuple(alloc.tensor_shape)
            dtype = mybir.dt.np(alloc.dtype)
            out_names.append(name)
            out_avals.append(jax.core.ShapedArray(shape, dtype))
            zero_shapes.append((shape, dtype))
    assert nc.dbg_addr is None, "runner assumes debug=False"
    assert in_names == ["wire"], in_names
    n_params = len(in_names)
    n_outs = len(out_avals)
    all_names = list(in_names) + list(out_names)
    if partition_name is not None:
        all_names.append(partition_name)
    donate = tuple(range(n_params, n_params + n_outs))

    def _body(*args):
        operands = list(args)
        if partition_name is not None:
            operands.append(bass2jax.partition_id_tensor())
        outs = bass2jax._bass_exec_p.bind(
            *operands,
            out_avals=tuple(out_avals),
            in_names=tuple(all_names),
            out_names=tuple(out_names),
            lowering_input_output_aliases=(),
            sim_require_finite=True,
            sim_require_nnan=True,
            nc=nc,
        )
        return tuple(outs)

    devices = jax.devices()[:N_CORES]
    mesh = Mesh(np.asarray(devices), ("core",))
    in_specs = (PartitionSpec("core"),) * (n_params + n_outs)
    out_specs = (PartitionSpec("core"),) * n_outs
    sharded = jax.jit(
        shard_map(_body, mesh=mesh, in_specs=in_specs, out_specs=out_specs,
                  check_rep=False),
        donate_argnums=donate, keep_unused=True)

    core_sharding = NamedSharding(mesh, PartitionSpec("core"))
    zeros_out = jax.jit(
        lambda: tuple(jnp.zeros((N_CORES * s[0], *s[1:]), d)
                      for s, d in zero_shapes),
        out_shardings=(core_sharding,) * n_outs)

    st = dict(sharded=sharded, devices=devices, core_sharding=core_sharding,
              zeros_out=zeros_out, out_names=out_names, zero_dev=None)
    _EXEC_CACHE[key] = st
    return st


def run_model(nc, wire0):
    """One full inference: ship wire0 to core 0, run, fetch output [B, T, H]."""
    st = _get_exec(nc)
    devices = st["devices"]
    if st["zero_dev"] is None:
        st["zero_dev"] = [
            jax.jit(lambda: jnp.zeros(wire0.shape, wire0.dtype),
                    out_shardings=SingleDeviceSharding(d))
            for d in devices[1:]]
    shard0 = jax.device_put(wire0, devices[0])
    shards = [shard0] + [zf() for zf in st["zero_dev"]]
    gshape = (N_CORES * wire0.shape[0],) + wire0.shape[1:]
    wire_g = jax.make_array_from_single_device_arrays(
        gshape, st["core_sharding"], shards)
    zouts = st["zeros_out"]()
    out_arrs = st["sharded"](wire_g, *zouts)
    res = np.asarray(out_arrs[0]).reshape(N_CORES, T // 2, H)
    out = np.empty((B, T, H), np.float32)
    for b in range(B):
        for h in range(2):
            out[b, h * (T // 2):(h + 1) * (T // 2), :] = res[b * 2 + h]
    return out


def kernel(**inputs):
    nc, wire0 = prepare(inputs)
    return run_model(nc, wire0)


# revision 15
# speedup vs baseline: 17.3899x; 1.1734x over previous
"""Trainium2 Bass kernel for nn_NeuralEncoder (sparse banded attention encoder).

Sharding: 8 cores = (batch b in 0..3) x (sequence half h in 0..1), uniform SPMD
program over a 1024-row local window per core: h=0 cores get 512 zero-pad rows +
rows 0..511, h=1 cores get rows 0..1023. Each layer shrinks the active window by
128 rows at the front (the CB=128 sliding-window halo); every core emits local
rows 512..1023 as its 512 output rows.

Host<->device transfer is the bottleneck (axon tunnel ~50MB/s, serialized), so
all inputs ship as ONE bf16 wire tensor to core 0 only; cores 1-7 receive
device-created zeros. On device an AllReduce(add) broadcasts the shared weight
blob and a ReduceScatter(add) hands each core its private window data
(spikes/rope tables/mask). Output returns in bf16.

Numerics: bf16 matmuls with fp32 PSUM accumulation; LayerNorm, softmax and the
residual stream in fp32. LN gains are folded into the following weight matrices
host-side; the band/padding/spikes_mask is a host-precomputed additive bias
applied to attention scores pre-exp.
"""

import os
import sys

for _p in ("/opt/trn_rl_repo", "/root/.axon_site/_ro/trn_rl_repo"):
    if _p not in sys.path and os.path.isdir(_p):
        sys.path.append(_p)

import numpy as np
import ml_dtypes
import jax
import jax.numpy as jnp
from jax.sharding import Mesh, PartitionSpec, NamedSharding, SingleDeviceSharding
try:
    from jax.experimental.shard_map import shard_map
except ImportError:
    from jax import shard_map

from concourse import bacc
import concourse.tile as tile
from concourse import mybir
from concourse import bass2jax
from concourse.masks import make_identity

# dims
B, T, C, D, H, NH, HD, INTER, L = 4, 1024, 256, 256, 512, 8, 64, 2048, 4
CF, CB, BASE = 0, 128, 10000.0
P = 128
NB = T // P          # 8 local row blocks
N_CORES = 8
NEG = np.float32(-1e30)
F32 = mybir.dt.float32
BF16 = mybir.dt.bfloat16
AF = mybir.ActivationFunctionType

_PROG_CACHE = {}
_EXEC_CACHE = {}


# ---------------------------------------------------------------------------
# wire layout (bf16 elems). Blob = broadcast (shared) region; PC = per-core.
# ---------------------------------------------------------------------------

def _blob_layout(has_bias):
    regions = [("rotm", P * P), ("band", P * 2 * P),
               ("embw", P * 512), ("projw", P * 1024)]
    for l in range(L):
        for nm in ("wq", "wk", "wv", "wo"):
            regions.append((f"{nm}{l}", P * 2048))
        regions.append((f"upw{l}", P * 8192))
        regions.append((f"dnw{l}", P * 8192))
    if has_bias:
        regions.append(("embb", P * 2))
        regions.append(("projb", H))
        for l in range(L):
            regions.append((f"bq{l}", P * 4))
            regions.append((f"bk{l}", P * 4))
            regions.append((f"bv{l}", H))
            regions.append((f"bo{l}", H))
            regions.append((f"upb{l}", P * 16))
            regions.append((f"dnb{l}", H))
    off, out = 0, {}
    for name, n in regions:
        out[name] = (off, n)
        off += n
    return out, off


# per-core region: offsets within each core's PCW-elem chunk
_PC_SPT = 0                    # [128, 2, 1024]
_PC_CST = P * 2048             # [32, 1024] (RoPE freqs repeat mod 32)
_PC_SNT = _PC_CST + 32 * 1024  # [32, 1024]
_PC_KIV = _PC_SNT + 32 * 1024  # [128, 8] additive key-invalid bias (pre-scaled)
PCW = _PC_KIV + P * NB


def _spans(start_block, end_block, max_blocks=4):
    """Split block range [start_block, end_block) into runs of <= max_blocks."""
    out = []
    b = start_block
    while b < end_block:
        e = min(b + max_blocks, end_block)
        out.append((b, e))
        b = e
    return out


def _build_program(has_bias):
    blob_off, blob_elems = _blob_layout(has_bias)
    nw = blob_elems + N_CORES * PCW

    nc = bacc.Bacc("TRN2", target_bir_lowering=False, debug=False,
                   num_devices=N_CORES)

    d_wire = nc.dram_tensor("wire", [nw], BF16, kind="ExternalInput")
    d_blob_in = nc.dram_tensor("blob_in", [blob_elems], BF16)
    d_blob = nc.dram_tensor("blob", [blob_elems], BF16, addr_space="Shared")
    d_pc_in = nc.dram_tensor("pc_in", [N_CORES * PCW], BF16)
    d_pc = nc.dram_tensor("pc", [PCW], BF16)
    d_olocal = nc.dram_tensor("olocal", [T // 2, H], BF16)
    d_og = nc.dram_tensor("og", [N_CORES * (T // 2), H], BF16, addr_space="Shared")
    d_out = nc.dram_tensor("out", [N_CORES * (T // 2), H], BF16,
                           kind="ExternalOutput")

    def bvw(name, pat, **dims):
        off, n = blob_off[name]
        ap = d_blob.ap()[off:off + n]
        return ap.rearrange(pat, **dims) if pat else ap

    def pcv(off, n, pat, **dims):
        ap = d_pc.ap()[off:off + n]
        return ap.rearrange(pat, **dims) if pat else ap

    with tile.TileContext(nc) as tc:
        with (
            tc.tile_pool(name="consts", bufs=1) as consts,
            tc.tile_pool(name="wts", bufs=2) as wts,
            tc.tile_pool(name="work", bufs=2) as work,
            tc.tile_pool(name="small", bufs=6) as small,
            tc.tile_pool(name="hTs", bufs=2) as hTs,
            tc.tile_pool(name="qk", bufs=1) as qk,
            tc.tile_pool(name="vp", bufs=9) as vp,
            tc.tile_pool(name="es", bufs=3) as es,
            tc.tile_pool(name="itp", bufs=1) as itp,
            tc.tile_pool(name="mm_ps", bufs=3, space="PSUM") as mm_ps,
            tc.tile_pool(name="s_ps", bufs=2, space="PSUM") as s_ps,
            tc.tile_pool(name="o_ps", bufs=2, space="PSUM") as o_ps,
            tc.tile_pool(name="t_ps", bufs=1, space="PSUM") as t_ps,
        ):
            # ---- distribute the wire: broadcast blob, scatter per-core ----
            nc.sync.dma_start(out=d_blob_in.ap(), in_=d_wire.ap()[0:blob_elems])
            nc.gpsimd.collective_compute(
                "AllReduce", mybir.AluOpType.add,
                replica_groups=[list(range(N_CORES))],
                ins=[d_blob_in.ap()], outs=[d_blob.ap()])
            nc.sync.dma_start(out=d_pc_in.ap(), in_=d_wire.ap()[blob_elems:nw])
            nc.gpsimd.collective_compute(
                "ReduceScatter", mybir.AluOpType.add,
                replica_groups=[list(range(N_CORES))],
                ins=[d_pc_in.ap()], outs=[d_pc.ap()])

            # ---- constants ----
            ident = consts.tile([P, P], BF16, tag="ident")
            make_identity(nc, ident[:])
            eps = consts.tile([P, 1], F32, tag="eps")
            nc.vector.memset(eps[:], 1e-5)
            csT = consts.tile([P, T], BF16, tag="csT")
            snT = consts.tile([P, T], BF16, tag="snT")
            for q in range(4):
                nc.sync.dma_start(out=csT[q * 32:(q + 1) * 32, :],
                                  in_=pcv(_PC_CST, 32 * T, "(p r) -> p r", p=32))
                nc.sync.dma_start(out=snT[q * 32:(q + 1) * 32, :],
                                  in_=pcv(_PC_SNT, 32 * T, "(p r) -> p r", p=32))
            kiv_b = consts.tile([P, NB], BF16, tag="kiv_b")
            nc.sync.dma_start(out=kiv_b[:],
                              in_=pcv(_PC_KIV, P * NB, "(p k) -> p k", p=P))
            kiv = consts.tile([P, NB], F32, tag="kiv")
            nc.scalar.activation(kiv[:], kiv_b[:], AF.Copy)
            band = consts.tile([P, 2 * P], BF16, tag="band")
            nc.sync.dma_start(out=band[:], in_=bvw("band", "(p q) -> p q", p=P))
            spT = consts.tile([P, C // P, T], BF16, tag="spT")
            nc.sync.dma_start(out=spT[:],
                              in_=pcv(_PC_SPT, P * 2 * T,
                                      "(p c r) -> p c r", p=P, c=C // P))
            rotm = consts.tile([P, P], BF16, tag="rotm")
            nc.sync.dma_start(out=rotm[:], in_=bvw("rotm", "(p m) -> p m", p=P))
            embw = consts.tile([P, C // P, D], BF16, tag="embw")
            nc.sync.dma_start(out=embw[:],
                              in_=bvw("embw", "(p c d) -> p c d", p=P, c=C // P))
            projw = consts.tile([P, D // P, H], BF16, tag="projw")
            nc.sync.dma_start(out=projw[:],
                              in_=bvw("projw", "(p c h) -> p c h", p=P, c=D // P))
            if has_bias:
                embb_b = consts.tile([P, D // P], BF16, tag="embb_b")
                nc.sync.dma_start(out=embb_b[:],
                                  in_=bvw("embb", "(p c) -> p c", p=P))
                embb = consts.tile([P, D // P], F32, tag="embb")
                nc.scalar.activation(embb[:], embb_b[:], AF.Copy)
                projb = consts.tile([1, H], BF16, tag="projb")
                nc.sync.dma_start(out=projb[:], in_=bvw("projb", "(a h) -> a h", a=1))
                ones_r = consts.tile([1, P], BF16, tag="ones_r")
                nc.vector.memset(ones_r[:], 1.0)

            x = consts.tile([P, NB, H], F32, tag="x")
            gT = consts.tile([P, D // P, T], BF16, tag="gT")

            def mm_group(ps, pairs, bias_row=None):
                """Accumulate lhsT.T @ rhs pairs into ps; optional bias row
                (psum += ones^T @ bias_row) closes the group."""
                for i, (a, bb) in enumerate(pairs):
                    last = (i == len(pairs) - 1) and bias_row is None
                    nc.tensor.matmul(ps, a, bb, start=(i == 0), stop=last)
                if bias_row is not None:
                    nc.tensor.matmul(ps, ones_r[:], bias_row,
                                     start=False, stop=True)

            # ---- embedding: gT = gelu(spikes @ embed_w)^T, x = gT^T @ proj_w ----
            for oc in range(D // P):
                for (s0, s1) in _spans(0, NB):
                    n = (s1 - s0) * P
                    ps = mm_ps.tile([P, 512], F32, tag="mm", name="mmps")[:, :n]
                    for fc in range(C // P):
                        nc.tensor.matmul(ps, embw[:, fc, oc * P:(oc + 1) * P],
                                         spT[:, fc, s0 * P:s0 * P + n],
                                         start=(fc == 0), stop=(fc == C // P - 1))
                    bias = embb[:, oc:oc + 1] if has_bias else 0.0
                    nc.scalar.activation(gT[:, oc, s0 * P:s0 * P + n], ps, AF.Gelu,
                                         bias=bias)
            for rb in range(NB):
                ps = mm_ps.tile([P, 512], F32, tag="mm")
                mm_group(ps,
                         [(gT[:, fc, rb * P:(rb + 1) * P], projw[:, fc, :])
                          for fc in range(D // P)],
                         bias_row=projb[:] if has_bias else None)
                nc.scalar.activation(x[:, rb, :], ps, AF.Copy)

            # ---- layers ----
            _trunc = os.environ.get("KTRUNC", "")
            n_layers = L
            if _trunc.startswith("L"):
                n_layers = int(_trunc[1:].split(":")[0])
            _phase = _trunc.split(":")[1] if ":" in _trunc else "all"
            for l in range(n_layers):
                kb0, qb0 = l, l + 1

                wq = wts.tile([P, H // P, H], BF16, tag="wq")
                nc.sync.dma_start(out=wq[:],
                                  in_=bvw(f"wq{l}", "(p f o) -> p f o", p=P, f=H // P))
                wk = wts.tile([P, H // P, H], BF16, tag="wk")
                nc.sync.dma_start(out=wk[:],
                                  in_=bvw(f"wk{l}", "(p f o) -> p f o", p=P, f=H // P))
                wv = wts.tile([P, H // P, H], BF16, tag="wv")
                nc.sync.dma_start(out=wv[:],
                                  in_=bvw(f"wv{l}", "(p f o) -> p f o", p=P, f=H // P))
                wo = wts.tile([P, H // P, H], BF16, tag="wo")
                nc.sync.dma_start(out=wo[:],
                                  in_=bvw(f"wo{l}", "(p f o) -> p f o", p=P, f=H // P))
                if has_bias:
                    bq_b = wts.tile([P, H // P], BF16, tag="bq_b")
                    nc.sync.dma_start(out=bq_b[:],
                                      in_=bvw(f"bq{l}", "(p c) -> p c", p=P))
                    bq = wts.tile([P, H // P], F32, tag="bq")
                    nc.scalar.activation(bq[:], bq_b[:], AF.Copy)
                    bk_b = wts.tile([P, H // P], BF16, tag="bk_b")
                    nc.sync.dma_start(out=bk_b[:],
                                      in_=bvw(f"bk{l}", "(p c) -> p c", p=P))
                    bk = wts.tile([P, H // P], F32, tag="bk")
                    nc.scalar.activation(bk[:], bk_b[:], AF.Copy)
                    bv = wts.tile([1, H], BF16, tag="bv")
                    nc.sync.dma_start(out=bv[:], in_=bvw(f"bv{l}", "(a h) -> a h", a=1))
                    bo = wts.tile([1, H], BF16, tag="bo")
                    nc.sync.dma_start(out=bo[:], in_=bvw(f"bo{l}", "(a h) -> a h", a=1))
                    dnb = wts.tile([1, H], BF16, tag="dnb")
                    nc.sync.dma_start(out=dnb[:],
                                      in_=bvw(f"dnb{l}", "(a h) -> a h", a=1))
                    upb_b = wts.tile([P, INTER // P], BF16, tag="upb_b")
                    nc.sync.dma_start(out=upb_b[:],
                                      in_=bvw(f"upb{l}", "(p c) -> p c", p=P))
                    upb = wts.tile([P, INTER // P], F32, tag="upb")
                    nc.scalar.activation(upb[:], upb_b[:], AF.Copy)

                def layernorm(src_ap, dst_bf16_ap):
                    stats = small.tile([P, 6], F32, tag="stats")
                    nc.vector.bn_stats(stats[:], src_ap)
                    mv = small.tile([P, 2], F32, tag="mv")
                    nc.vector.bn_aggr(mv[:], stats[:])
                    rstd = small.tile([P, 1], F32, tag="rstd")
                    nc.scalar.activation(rstd[:], mv[:, 1:2], AF.Sqrt, bias=eps[:])
                    nc.vector.reciprocal(rstd[:], rstd[:])
                    nc.vector.tensor_scalar(dst_bf16_ap, src_ap,
                                            mv[:, 0:1], rstd[:],
                                            mybir.AluOpType.subtract,
                                            mybir.AluOpType.mult)

                def transpose128(src_bf16_ap, dst_bf16_ap):
                    # src [128, 128] -> dst [128, 128] via PE transpose
                    tp = t_ps.tile([P, P], BF16, tag="tp")
                    nc.tensor.transpose(tp[:], src_bf16_ap, ident[:])
                    nc.scalar.activation(dst_bf16_ap, tp[:], AF.Copy)

                # LN1 + h^T + v for key range
                hT = hTs.tile([P, H // P, T], BF16, tag="hT")
                vtiles = {}
                for kb in range(kb0, NB):
                    hrow = work.tile([P, H], BF16, tag="hrow")
                    layernorm(x[:, kb, :], hrow[:])
                    for fc in range(H // P):
                        transpose128(hrow[:, fc * P:(fc + 1) * P],
                                     hT[:, fc, kb * P:(kb + 1) * P])
                    ps = mm_ps.tile([P, 512], F32, tag="mm")
                    mm_group(ps,
                             [(hT[:, fc, kb * P:(kb + 1) * P], wv[:, fc, :])
                              for fc in range(H // P)],
                             bias_row=bv[:] if has_bias else None)
                    vt = vp.tile([P, NH, HD + 1], BF16, tag="v")
                    nc.scalar.activation(vt[:, :, 0:HD],
                                         ps.rearrange("p (h d) -> p h d", h=NH),
                                         AF.Copy)
                    nc.vector.memset(vt[:, :, HD:HD + 1], 1.0)
                    vtiles[kb] = vt

                if _phase == "v" and l == n_layers - 1:
                    continue
                # q^T / k^T with RoPE
                qT = qk.tile([P, H // P, T], BF16, tag="qT")
                kT = qk.tile([P, H // P, T], BF16, tag="kT")
                for (dst, w, bias_t, blk0) in (
                    (qT, wq, "bq", qb0),
                    (kT, wk, "bk", kb0),
                ):
                    for oc in range(H // P):
                        for (s0, s1) in _spans(blk0, NB):
                            n = (s1 - s0) * P
                            c0 = s0 * P
                            ps = mm_ps.tile([P, 512], F32, tag="mm", name="mmps")[:, :n]
                            for fc in range(H // P):
                                nc.tensor.matmul(ps, w[:, fc, oc * P:(oc + 1) * P],
                                                 hT[:, fc, c0:c0 + n],
                                                 start=(fc == 0),
                                                 stop=(fc == H // P - 1))
                            q0 = work.tile([P, 512], BF16, tag="q0", name="q0t")[:, :n]
                            if has_bias:
                                bt = bq if bias_t == "bq" else bk
                                nc.scalar.activation(q0, ps, AF.Copy,
                                                     bias=bt[:, oc:oc + 1])
                            else:
                                nc.scalar.activation(q0, ps, AF.Copy)
                            # rope: out = q0 * cs + rot_half(q0) * sn,
                            # rot_half via signed-permutation matmul on PE
                            rp = mm_ps.tile([P, 512], F32, tag="mm", name="rpps")[:, :n]
                            nc.tensor.matmul(rp, rotm[:], q0, start=True, stop=True)
                            t1 = work.tile([P, 512], BF16, tag="t1", name="t1t")[:, :n]
                            nc.vector.tensor_mul(t1, rp, snT[:, c0:c0 + n])
                            t2 = work.tile([P, 512], BF16, tag="t2", name="t2t")[:, :n]
                            nc.vector.tensor_mul(t2, q0, csT[:, c0:c0 + n])
                            nc.vector.tensor_add(dst[:, oc, c0:c0 + n], t1, t2)

                if _phase == "qk" and l == n_layers - 1:
                    continue
                # scores + exp per (kb), then PV/Wo for qb == kb
                estiles = {}
                for kb in range(kb0, NB):
                    qlo, qhi = max(kb, qb0), min(kb + 2, NB)
                    n = (qhi - qlo) * P
                    c0 = qlo * P
                    moff = (qlo - kb) * P
                    for h in range(NH):
                        hp0 = 64 * (h % 2)
                        hc = h // 2
                        sp = s_ps.tile([P, 2 * P], F32, tag="s", name="spt")[:, :n]
                        nc.tensor.matmul(sp,
                                         kT[hp0:hp0 + 64, hc, kb * P:(kb + 1) * P],
                                         qT[hp0:hp0 + 64, hc, c0:c0 + n],
                                         start=True, stop=True)
                        nc.vector.tensor_add(sp, sp, band[:, moff:moff + n])
                        est = es.tile([P, 2 * P], BF16, tag=f"es{h}")
                        nc.scalar.activation(est[:, moff:moff + n], sp, AF.Exp,
                                             scale=0.125, bias=kiv[:, kb:kb + 1])
                        estiles[(h, kb)] = est

                    if kb < qb0 or _phase == "scores":
                        continue
                    qb = kb
                    # PV with appended-ones denominator column
                    ops_ = [o_ps.tile([P, 4, HD + 1], F32, tag="o", name=f"opst{_g}") for _g in range(2)]
                    for h in range(NH):
                        sl = ops_[h // 4][:, h % 4, :]
                        nc.tensor.matmul(sl, estiles[(h, qb)][:, 0:P],
                                         vtiles[qb][:, h, :], start=True, stop=False)
                        nc.tensor.matmul(sl, estiles[(h, qb - 1)][:, P:2 * P],
                                         vtiles[qb - 1][:, h, :], start=False, stop=True)
                    if _phase == "pv1":
                        continue
                    den = small.tile([P, NH], F32, tag="den")
                    nc.scalar.activation(den[:, 0:4], ops_[0][:, :, HD], AF.Copy,
                                         bias=1e-20)
                    nc.scalar.activation(den[:, 4:8], ops_[1][:, :, HD], AF.Copy,
                                         bias=1e-20)
                    nc.vector.reciprocal(den[:], den[:])
                    if _phase == "pv2":
                        continue
                    osc = work.tile([P, H], BF16, tag="osc")
                    for g in range(2):
                        nc.vector.tensor_mul(
                            osc.rearrange("p (g2 h d) -> p g2 h d", g2=2, h=4)[:, g],
                            ops_[g][:, :, 0:HD],
                            den[:, g * 4:(g + 1) * 4, None].to_broadcast((P, 4, HD)))
                    if _phase == "pv":
                        continue
                    oT = work.tile([P, H // P, P], BF16, tag="oT")
                    for fc in range(H // P):
                        transpose128(osc[:, fc * P:(fc + 1) * P], oT[:, fc, :])
                    ps = mm_ps.tile([P, 512], F32, tag="mm")
                    mm_group(ps,
                             [(oT[:, fc, :], wo[:, fc, :]) for fc in range(H // P)],
                             bias_row=bo[:] if has_bias else None)
                    nc.vector.tensor_add(x[:, qb, :], ps, x[:, qb, :])

                if _phase == "attn" and l == n_layers - 1:
                    continue
                # ---- MLP ----
                h2T = hTs.tile([P, H // P, T], BF16, tag="hT")
                for qb in range(qb0, NB):
                    hrow = work.tile([P, H], BF16, tag="hrow")
                    layernorm(x[:, qb, :], hrow[:])
                    for fc in range(H // P):
                        transpose128(hrow[:, fc * P:(fc + 1) * P],
                                     h2T[:, fc, qb * P:(qb + 1) * P])

                for (s0, s1) in _spans(qb0, NB):
                    n = (s1 - s0) * P
                    c0 = s0 * P
                    it = itp.tile([P, INTER // P, 512], BF16, tag="iT")
                    for icg in range(2):
                        uw = wts.tile([P, H // P, INTER // 2], BF16, tag="upw")
                        nc.sync.dma_start(
                            out=uw[:],
                            in_=bvw(f"upw{l}", "(p f i) -> p f i", p=P, f=H // P)[
                                :, :, icg * (INTER // 2):(icg + 1) * (INTER // 2)])
                        for ic in range(INTER // 2 // P):
                            icx = icg * (INTER // 2 // P) + ic
                            ps = mm_ps.tile([P, 512], F32, tag="mm", name="mmps")[:, :n]
                            for fc in range(H // P):
                                nc.tensor.matmul(ps, uw[:, fc, ic * P:(ic + 1) * P],
                                                 h2T[:, fc, c0:c0 + n],
                                                 start=(fc == 0),
                                                 stop=(fc == H // P - 1))
                            bias = upb[:, icx:icx + 1] if has_bias else 0.0
                            nc.scalar.activation(it[:, icx, :n], ps, AF.Gelu,
                                                 bias=bias)
                    dw = [None, None]
                    for icg in range(2):
                        dw[icg] = wts.tile([P, INTER // 2 // P, H], BF16, tag="dnw",
                                           name=f"dnw{icg}")
                        nc.sync.dma_start(
                            out=dw[icg][:],
                            in_=bvw(f"dnw{l}", "(p g o) -> p g o", p=P, g=INTER // P)[
                                :, icg * (INTER // 2 // P):(icg + 1) * (INTER // 2 // P), :])
                    for qb in range(s0, s1):
                        rel = (qb - s0) * P
                        ps = mm_ps.tile([P, 512], F32, tag="mm")
                        mm_group(ps,
                                 [(it[:, icx, rel:rel + P], dw[icx // 8][:, icx % 8, :])
                                  for icx in range(INTER // P)],
                                 bias_row=dnb[:] if has_bias else None)
                        nc.vector.tensor_add(x[:, qb, :], ps, x[:, qb, :])

            # ---- output: local blocks 4..8, bf16, gathered onto every core ----
            xout = consts.tile([P, NB // 2, H], BF16, tag="xout")
            nc.scalar.activation(xout[:], x[:, NB // 2:NB, :], AF.Copy)
            nc.sync.dma_start(
                out=d_olocal.ap().rearrange("(b p) h -> p b h", p=P),
                in_=xout[:])
            nc.gpsimd.collective_compute(
                "AllGather", mybir.AluOpType.bypass,
                replica_groups=[list(range(N_CORES))],
                ins=[d_olocal.ap()], outs=[d_og.ap()])
            nc.sync.dma_start(out=d_out.ap(), in_=d_og.ap())

    nc.finalize()
    return nc


def _rope_tables():
    inv = 1.0 / (BASE ** (np.arange(0, HD, 2, dtype=np.float32) / np.float32(HD)))
    t = np.arange(T, dtype=np.float32)
    f = t[:, None] * inv[None, :]                      # [T, HD/2]
    emb = np.concatenate([f, f], axis=-1)              # [T, HD]
    return np.cos(emb).astype(np.float32), np.sin(emb).astype(np.float32)


def _bf16(x):
    return np.ascontiguousarray(np.asarray(x, np.float32)).astype(ml_dtypes.bfloat16)


def _pmajor(w, p_groups):
    """[G*128, X] row-major -> [128, G, X] p-major, raveled (bf16)."""
    w = np.asarray(w)
    g = w.shape[0] // P
    return _bf16(w.reshape(g, P, -1).transpose(1, 0, 2)).ravel()


def prepare(inputs):
    """Host-side preprocessing: returns (nc, wire0) — wire0 ships to core 0."""
    inp = {k: np.asarray(v) for k, v in inputs.items()}
    spikes = inp["spikes"].astype(np.float32)          # [B, T, C]
    spikes_mask = inp["spikes_mask"].astype(np.int32)  # [B, T]
    ts = inp["spikes_timestamp"].astype(np.int64)      # [B, T]

    # ---- fold LN gains/biases into weights host-side ----
    ln1_g, ln1_b = inp["ln1_g"].astype(np.float32), inp["ln1_b"].astype(np.float32)
    ln2_g, ln2_b = inp["ln2_g"].astype(np.float32), inp["ln2_b"].astype(np.float32)
    Wq, Wk, Wv, Wo = (inp[k].astype(np.float32) for k in ("Wq", "Wk", "Wv", "Wo"))
    upw, dnw = inp["up_w"].astype(np.float32), inp["down_w"].astype(np.float32)
    bq = inp["bq"].astype(np.float32) + np.einsum("lh,lho->lo", ln1_b, Wq)
    bk = inp["bk"].astype(np.float32) + np.einsum("lh,lho->lo", ln1_b, Wk)
    bv = inp["bv"].astype(np.float32) + np.einsum("lh,lho->lo", ln1_b, Wv)
    bo = inp["bo"].astype(np.float32)
    upb = inp["up_b"].astype(np.float32) + np.einsum("lh,lhi->li", ln2_b, upw)
    dnb = inp["down_b"].astype(np.float32)
    wq_eff = ln1_g[:, :, None] * Wq
    wk_eff = ln1_g[:, :, None] * Wk
    wv_eff = ln1_g[:, :, None] * Wv
    upw_eff = ln2_g[:, :, None] * upw

    has_bias = bool(
        np.abs(inp["embed_b"]).max() > 0 or np.abs(inp["proj_b"]).max() > 0
        or max(np.abs(a).max() for a in (bq, bk, bv, bo, upb, dnb)) > 0)

    key = has_bias
    if key not in _PROG_CACHE:
        _PROG_CACHE[key] = _build_program(has_bias)
    nc = _PROG_CACHE[key]

    blob_off, blob_elems = _blob_layout(has_bias)

    # signed permutation for rotate-half: out[m] = sign(m) * q[partner(m)]
    # (as matmul rotm.T @ q: rotm[partner(m), m] = sign(m))
    rotm_np = np.zeros((P, P), np.float32)
    for m in range(P):
        d = m % HD
        partner = m + HD // 2 if d < HD // 2 else m - HD // 2
        rotm_np[partner, m] = -1.0 if d < HD // 2 else 1.0

    blob = np.zeros(blob_elems, ml_dtypes.bfloat16)

    def put(name, arr_flat):
        off, n = blob_off[name]
        assert arr_flat.size == n, (name, arr_flat.size, n)
        blob[off:off + n] = arr_flat

    put("rotm", _bf16(rotm_np).ravel())
    # band structure in local coords, shared by all cores and key blocks:
    # col block 0 (q in same block as k): allow kc <= qc; col block 1
    # (q one block above k): allow kc >= qc.
    kc_ = np.arange(P)[:, None]
    qc_ = np.arange(P)[None, :]
    band_np = np.concatenate(
        [np.where(kc_ <= qc_, 0.0, NEG), np.where(kc_ >= qc_, 0.0, NEG)],
        axis=1).astype(np.float32)
    put("band", _bf16(band_np).ravel())
    put("embw", _pmajor(inp["embed_w"], 2))
    put("projw", _pmajor(inp["proj_w"], 2))
    for l in range(L):
        put(f"wq{l}", _pmajor(wq_eff[l], 4))
        put(f"wk{l}", _pmajor(wk_eff[l], 4))
        put(f"wv{l}", _pmajor(wv_eff[l], 4))
        put(f"wo{l}", _pmajor(Wo[l], 4))
        put(f"upw{l}", _pmajor(upw_eff[l], 4))
        put(f"dnw{l}", _pmajor(dnw[l], 16))
    if has_bias:
        put("embb", _bf16(inp["embed_b"].reshape(2, P).T).ravel())
        put("projb", _bf16(inp["proj_b"]).ravel())
        for l in range(L):
            put(f"bq{l}", _bf16(bq[l].reshape(4, P).T).ravel())
            put(f"bk{l}", _bf16(bk[l].reshape(4, P).T).ravel())
            put(f"bv{l}", _bf16(bv[l]).ravel())
            put(f"bo{l}", _bf16(bo[l]).ravel())
            put(f"upb{l}", _bf16(upb[l].reshape(16, P).T).ravel())
            put(f"dnb{l}", _bf16(dnb[l]).ravel())

    cos_t, sin_t = _rope_tables()   # [T, HD]

    pcs = []
    for b in range(B):
        for h in range(2):
            g0 = h * (T // 2)       # global row of local row 512
            # local row r -> global row r - 512 + g0
            gl = np.arange(T) - (T // 2) + g0
            valid = gl >= 0
            glc = np.clip(gl, 0, T - 1)

            spT_local = np.zeros((C, T), np.float32)
            spT_local[:, valid] = spikes[b, glc[valid], :].T

            ts_local = np.where(valid, ts[b, glc], 0)
            cs_l = cos_t[ts_local]          # [T(local), HD]
            sn_l = sin_t[ts_local]
            # feature-major rope tables [32, T] (freqs repeat mod HD/2=32)
            csT_l = cs_l[:, 0:HD // 2].T.astype(np.float32)       # [32, T]
            snT_l = sn_l[:, 0:HD // 2].T.astype(np.float32)

            # per-key additive invalid bias [kc, kb], pre-scaled by 0.125
            # (rides the Exp activation's per-partition bias column)
            gk = (np.arange(T) - (T // 2) + g0)
            kvalid = (gk >= 0) & (spikes_mask[b, np.clip(gk, 0, T - 1)] > 0)
            kiv = np.where(kvalid, 0.0, NEG * 0.125).astype(np.float32)
            kiv = kiv.reshape(NB, P).T                            # [128, NB]

            pc = np.empty(PCW, ml_dtypes.bfloat16)
            pc[_PC_SPT:_PC_SPT + P * 2 * T] = _pmajor(spT_local, 2)
            pc[_PC_CST:_PC_CST + 32 * T] = _bf16(csT_l).ravel()
            pc[_PC_SNT:_PC_SNT + 32 * T] = _bf16(snT_l).ravel()
            pc[_PC_KIV:_PC_KIV + P * NB] = _bf16(kiv).ravel()
            pcs.append(pc)

    wire0 = np.concatenate([blob] + pcs)
    return nc, wire0


# ---------------------------------------------------------------------------
# cached-jit runner: wire ships to core 0 only; cores 1..7 get device zeros
# ---------------------------------------------------------------------------

def _get_exec(nc):
    key = id(nc)
    if key in _EXEC_CACHE:
        return _EXEC_CACHE[key]
    bass2jax.install_neuronx_cc_hook()
    partition_name = nc.partition_id_tensor.name if nc.partition_id_tensor else None
    in_names, out_names, out_avals, zero_shapes = [], [], [], []
    for alloc in nc.m.functions[0].allocations:
        if not isinstance(alloc, mybir.MemoryLocationSet):
            continue
        name = alloc.memorylocations[0].name
        if alloc.kind == "ExternalInput":
            if name != partition_name:
                in_names.append(name)
        elif alloc.kind == "ExternalOutput":
            shape = tuple(alloc.tensor_shape)
            dtype = mybir.dt.np(alloc.dtype)
            out_names.append(name)
            out_avals.append(jax.core.ShapedArray(shape, dtype))
            zero_shapes.append((shape, dtype))
    assert nc.dbg_addr is None, "runner assumes debug=False"
    assert in_names == ["wire"], in_names
    n_params = len(in_names)
    n_outs = len(out_avals)
    all_names = list(in_names) + list(out_names)
    if partition_name is not None:
        all_names.append(partition_name)
    donate = tuple(range(n_params, n_params + n_outs))

    def _body(*args):
        operands = list(args)
        if partition_name is not None:
            operands.append(bass2jax.partition_id_tensor())
        outs = bass2jax._bass_exec_p.bind(
            *operands,
            out_avals=tuple(out_avals),
            in_names=tuple(all_names),
            out_names=tuple(out_names),
            lowering_input_output_aliases=(),
            sim_require_finite=True,
            sim_require_nnan=True,
            nc=nc,
        )
        return tuple(outs)

    devices = jax.devices()[:N_CORES]
    mesh = Mesh(np.asarray(devices), ("core",))
    in_specs = (PartitionSpec("core"),) * (n_params + n_outs)
    out_specs = (PartitionSpec("core"),) * n_outs
    sharded = jax.jit(
        shard_map(_body, mesh=mesh, in_specs=in_specs, out_specs=out_specs,
                  check_rep=False),
        donate_argnums=donate, keep_unused=True)

    core_sharding = NamedSharding(mesh, PartitionSpec("core"))
    zeros_out = jax.jit(
        lambda: tuple(jnp.zeros((N_CORES * s[0], *s[1:]), d)
                      for s, d in zero_shapes),
        out_shardings=(core_sharding,) * n_outs)

    st = dict(sharded=sharded, devices=devices, core_sharding=core_sharding,
              zeros_out=zeros_out, out_names=out_names, zero_dev=None)
    _EXEC_CACHE[key] = st
    return st


def run_model(nc, wire0):
    """One full inference: ship wire0 to core 0, run, fetch output [B, T, H]."""
    st = _get_exec(nc)
    devices = st["devices"]
    if st["zero_dev"] is None:
        # persistent zero padding shards for cores 1..7 (inputs, not donated)
        st["zero_dev"] = [
            jax.jit(lambda: jnp.zeros(wire0.shape, wire0.dtype),
                    out_shardings=SingleDeviceSharding(d))()
            for d in devices[1:]]
    shard0 = jax.device_put(wire0, devices[0])
    shards = [shard0] + st["zero_dev"]
    gshape = (N_CORES * wire0.shape[0],) + wire0.shape[1:]
    wire_g = jax.make_array_from_single_device_arrays(
        gshape, st["core_sharding"], shards)
    zouts = st["zeros_out"]()
    out_arrs = st["sharded"](wire_g, *zouts)
    # output was AllGathered on-device: every core holds the full result, so
    # fetch only core 0's shard (one d2h transfer)
    res = np.asarray(out_arrs[0].addressable_shards[0].data)
    res = res.reshape(N_CORES, T // 2, H)
    out = np.empty((B, T, H), np.float32)
    for b in range(B):
        for h in range(2):
            out[b, h * (T // 2):(h + 1) * (T // 2), :] = res[b * 2 + h]
    return out


def kernel(**inputs):
    nc, wire0 = prepare(inputs)
    return run_model(nc, wire0)


# revision 22
# speedup vs baseline: 19.0347x; 1.0946x over previous
"""Trainium2 Bass kernel for nn_NeuralEncoder (sparse banded attention encoder).

Sharding: 8 cores = (batch b in 0..3) x (sequence half h in 0..1), uniform SPMD
program over a 1024-row local window per core: h=0 cores get 512 zero-pad rows +
rows 0..511, h=1 cores get rows 0..1023. Each layer shrinks the active window by
128 rows at the front (the CB=128 sliding-window halo); every core emits local
rows 512..1023 as its 512 output rows.

Host<->device transfer is the bottleneck (axon tunnel ~50MB/s, serialized), so
all inputs ship as ONE bf16 wire tensor to core 0 only; cores 1-7 receive
device-created zeros. On device an AllReduce(add) broadcasts the shared weight
blob and a ReduceScatter(add) hands each core its private window data
(spikes/rope tables/mask). Output returns in bf16.

Numerics: bf16 matmuls with fp32 PSUM accumulation; LayerNorm, softmax and the
residual stream in fp32. LN gains are folded into the following weight matrices
host-side; the band/padding/spikes_mask is a host-precomputed additive bias
applied to attention scores pre-exp.
"""

import os
import sys

for _p in ("/opt/trn_rl_repo", "/root/.axon_site/_ro/trn_rl_repo"):
    if _p not in sys.path and os.path.isdir(_p):
        sys.path.append(_p)

import numpy as np
import ml_dtypes
import jax
import jax.numpy as jnp
from jax.sharding import Mesh, PartitionSpec, NamedSharding, SingleDeviceSharding
try:
    from jax.experimental.shard_map import shard_map
except ImportError:
    from jax import shard_map

from concourse import bacc
import concourse.tile as tile
from concourse import mybir
from concourse import bass2jax
from concourse.masks import make_identity

# dims
B, T, C, D, H, NH, HD, INTER, L = 4, 1024, 256, 256, 512, 8, 64, 2048, 4
CF, CB, BASE = 0, 128, 10000.0
P = 128
NB = T // P          # 8 local row blocks
N_CORES = 8
NEG = np.float32(-1e30)
F32 = mybir.dt.float32
BF16 = mybir.dt.bfloat16
AF = mybir.ActivationFunctionType

_PROG_CACHE = {}
_EXEC_CACHE = {}


# ---------------------------------------------------------------------------
# wire layout (bf16 elems). Blob = broadcast (shared) region; PC = per-core.
# ---------------------------------------------------------------------------

def _blob_layout(has_bias):
    regions = [("rotm", P * P), ("band", P * 2 * P),
               ("embw", P * 512), ("projw", P * 1024)]
    for l in range(L):
        for nm in ("wq", "wk", "wv", "wo"):
            regions.append((f"{nm}{l}", P * 2048))
        regions.append((f"upw{l}", P * 8192))
        regions.append((f"dnw{l}", P * 8192))
    if has_bias:
        regions.append(("embb", P * 2))
        regions.append(("projb", H))
        for l in range(L):
            regions.append((f"bq{l}", P * 4))
            regions.append((f"bk{l}", P * 4))
            regions.append((f"bv{l}", H))
            regions.append((f"bo{l}", H))
            regions.append((f"upb{l}", P * 16))
            regions.append((f"dnb{l}", H))
    off, out = 0, {}
    for name, n in regions:
        out[name] = (off, n)
        off += n
    return out, off


# per-core region: offsets within each core's PCW-elem chunk. Each core ships
# only its OWN 512 rows of spikes/rope tables; the 512-row halo comes from the
# (h=0, h=1) pair partner via an on-device pair-wise AllGather of the first
# _PC_PAIR elems, zeroed for h=0 cores by the halo flag.
_PC_SPT = 0                      # [128, 2, 512] own spikes.T (local cols 512:1024)
_PC_CST = P * 2 * 512            # [32, 512] own cos (RoPE freqs repeat mod 32)
_PC_SNT = _PC_CST + 32 * 512     # [32, 512] own sin
_PC_PAIR = _PC_SNT + 32 * 512    # pair-AllGathered prefix ends here
_PC_KIV = _PC_PAIR               # [128, 8] additive key-invalid bias (pre-scaled)
_PC_FLG = _PC_KIV + P * NB       # [128, 1] halo flag (h=1 -> 1.0, h=0 -> 0.0)
PCW = _PC_FLG + P


def _spans(start_block, end_block, max_blocks=4):
    """Split block range [start_block, end_block) into runs of <= max_blocks."""
    out = []
    b = start_block
    while b < end_block:
        e = min(b + max_blocks, end_block)
        out.append((b, e))
        b = e
    return out


def _build_program(has_bias):
    blob_off, blob_elems = _blob_layout(has_bias)
    nw = blob_elems + N_CORES * PCW

    nc = bacc.Bacc("TRN2", target_bir_lowering=False, debug=False,
                   num_devices=N_CORES)

    d_wire = nc.dram_tensor("wire", [nw], BF16, kind="ExternalInput")
    d_blob_in = nc.dram_tensor("blob_in", [blob_elems], BF16)
    d_blob = nc.dram_tensor("blob", [blob_elems], BF16, addr_space="Shared")
    d_pc_in = nc.dram_tensor("pc_in", [N_CORES * PCW], BF16)
    d_pc = nc.dram_tensor("pc", [PCW], BF16)
    d_pair = nc.dram_tensor("pair", [2 * _PC_PAIR], BF16)
    d_olocal = nc.dram_tensor("olocal", [T // 2, H], BF16)
    d_og = nc.dram_tensor("og", [N_CORES * (T // 2), H], BF16, addr_space="Shared")
    d_out = nc.dram_tensor("out", [N_CORES * (T // 2), H], BF16,
                           kind="ExternalOutput")

    def bvw(name, pat, **dims):
        off, n = blob_off[name]
        ap = d_blob.ap()[off:off + n]
        return ap.rearrange(pat, **dims) if pat else ap

    def pcv(off, n, pat, **dims):
        ap = d_pc.ap()[off:off + n]
        return ap.rearrange(pat, **dims) if pat else ap

    with tile.TileContext(nc) as tc:
        with (
            tc.tile_pool(name="consts", bufs=1) as consts,
            tc.tile_pool(name="wts", bufs=2) as wts,
            tc.tile_pool(name="work", bufs=2) as work,
            tc.tile_pool(name="small", bufs=6) as small,
            tc.tile_pool(name="hTs", bufs=2) as hTs,
            tc.tile_pool(name="qk", bufs=1) as qk,
            tc.tile_pool(name="vp", bufs=9) as vp,
            tc.tile_pool(name="es", bufs=3) as es,
            tc.tile_pool(name="itp", bufs=1) as itp,
            tc.tile_pool(name="mm_ps", bufs=3, space="PSUM") as mm_ps,
            tc.tile_pool(name="s_ps", bufs=2, space="PSUM") as s_ps,
            tc.tile_pool(name="o_ps", bufs=2, space="PSUM") as o_ps,
            tc.tile_pool(name="t_ps", bufs=1, space="PSUM") as t_ps,
        ):
            # ---- distribute the wire: broadcast blob, scatter per-core ----
            nc.sync.dma_start(out=d_blob_in.ap(), in_=d_wire.ap()[0:blob_elems])
            nc.gpsimd.collective_compute(
                "AllReduce", mybir.AluOpType.add,
                replica_groups=[list(range(N_CORES))],
                ins=[d_blob_in.ap()], outs=[d_blob.ap()])
            nc.sync.dma_start(out=d_pc_in.ap(), in_=d_wire.ap()[blob_elems:nw])
            nc.gpsimd.collective_compute(
                "ReduceScatter", mybir.AluOpType.add,
                replica_groups=[list(range(N_CORES))],
                ins=[d_pc_in.ap()], outs=[d_pc.ap()])
            # halo exchange within (h=0, h=1) pairs: chunk 0 of d_pair is the
            # even core's block = the batch's global rows [0, 512)
            nc.gpsimd.collective_compute(
                "AllGather", mybir.AluOpType.bypass,
                replica_groups=[[2 * i, 2 * i + 1] for i in range(N_CORES // 2)],
                ins=[d_pc.ap()[0:_PC_PAIR]], outs=[d_pair.ap()])

            # ---- constants ----
            ident = consts.tile([P, P], BF16, tag="ident")
            make_identity(nc, ident[:])
            eps = consts.tile([P, 1], F32, tag="eps")
            nc.vector.memset(eps[:], 1e-5)
            def prv(off, n, pat, **dims):
                ap = d_pair.ap()[off:off + n]
                return ap.rearrange(pat, **dims) if pat else ap

            TH = T // 2
            csT = consts.tile([P, T], BF16, tag="csT")
            snT = consts.tile([P, T], BF16, tag="snT")
            for q in range(4):
                r0, r1 = q * 32, (q + 1) * 32
                nc.sync.dma_start(out=csT[r0:r1, 0:TH],
                                  in_=prv(_PC_CST, 32 * TH, "(p r) -> p r", p=32))
                nc.sync.dma_start(out=csT[r0:r1, TH:T],
                                  in_=pcv(_PC_CST, 32 * TH, "(p r) -> p r", p=32))
                nc.sync.dma_start(out=snT[r0:r1, 0:TH],
                                  in_=prv(_PC_SNT, 32 * TH, "(p r) -> p r", p=32))
                nc.sync.dma_start(out=snT[r0:r1, TH:T],
                                  in_=pcv(_PC_SNT, 32 * TH, "(p r) -> p r", p=32))
            kiv_b = consts.tile([P, NB], BF16, tag="kiv_b")
            nc.sync.dma_start(out=kiv_b[:],
                              in_=pcv(_PC_KIV, P * NB, "(p k) -> p k", p=P))
            kiv = consts.tile([P, NB], F32, tag="kiv")
            nc.scalar.activation(kiv[:], kiv_b[:], AF.Copy)
            hflag_b = consts.tile([P, 1], BF16, tag="hflag_b")
            nc.sync.dma_start(out=hflag_b[:], in_=pcv(_PC_FLG, P, "(p a) -> p a", p=P))
            hflag = consts.tile([P, 1], F32, tag="hflag")
            nc.scalar.activation(hflag[:], hflag_b[:], AF.Copy)
            band = consts.tile([P, 2 * P], BF16, tag="band")
            nc.sync.dma_start(out=band[:], in_=bvw("band", "(p q) -> p q", p=P))
            spT = consts.tile([P, C // P, T], BF16, tag="spT")
            nc.sync.dma_start(out=spT[:, :, TH:T],
                              in_=pcv(_PC_SPT, P * 2 * TH,
                                      "(p c r) -> p c r", p=P, c=C // P))
            nc.sync.dma_start(out=spT[:, :, 0:TH],
                              in_=prv(_PC_SPT, P * 2 * TH,
                                      "(p c r) -> p c r", p=P, c=C // P))
            # zero the halo on h=0 cores (their pair-chunk 0 is their own data)
            nc.vector.tensor_scalar(spT[:, :, 0:TH], spT[:, :, 0:TH],
                                    hflag[:], None, mybir.AluOpType.mult)
            rotm = consts.tile([P, P], BF16, tag="rotm")
            nc.sync.dma_start(out=rotm[:], in_=bvw("rotm", "(p m) -> p m", p=P))
            embw = consts.tile([P, C // P, D], BF16, tag="embw")
            nc.sync.dma_start(out=embw[:],
                              in_=bvw("embw", "(p c d) -> p c d", p=P, c=C // P))
            projw = consts.tile([P, D // P, H], BF16, tag="projw")
            nc.sync.dma_start(out=projw[:],
                              in_=bvw("projw", "(p c h) -> p c h", p=P, c=D // P))
            if has_bias:
                embb_b = consts.tile([P, D // P], BF16, tag="embb_b")
                nc.sync.dma_start(out=embb_b[:],
                                  in_=bvw("embb", "(p c) -> p c", p=P))
                embb = consts.tile([P, D // P], F32, tag="embb")
                nc.scalar.activation(embb[:], embb_b[:], AF.Copy)
                projb = consts.tile([1, H], BF16, tag="projb")
                nc.sync.dma_start(out=projb[:], in_=bvw("projb", "(a h) -> a h", a=1))
                ones_r = consts.tile([1, P], BF16, tag="ones_r")
                nc.vector.memset(ones_r[:], 1.0)

            x = consts.tile([P, NB, H], F32, tag="x")
            gT = consts.tile([P, D // P, T], BF16, tag="gT")

            def mm_group(ps, pairs, bias_row=None):
                """Accumulate lhsT.T @ rhs pairs into ps; optional bias row
                (psum += ones^T @ bias_row) closes the group."""
                for i, (a, bb) in enumerate(pairs):
                    last = (i == len(pairs) - 1) and bias_row is None
                    nc.tensor.matmul(ps, a, bb, start=(i == 0), stop=last)
                if bias_row is not None:
                    nc.tensor.matmul(ps, ones_r[:], bias_row,
                                     start=False, stop=True)

            # ---- embedding: gT = gelu(spikes @ embed_w)^T, x = gT^T @ proj_w ----
            for oc in range(D // P):
                for (s0, s1) in _spans(0, NB):
                    n = (s1 - s0) * P
                    ps = mm_ps.tile([P, 512], F32, tag="mm", name="mmps")[:, :n]
                    for fc in range(C // P):
                        nc.tensor.matmul(ps, embw[:, fc, oc * P:(oc + 1) * P],
                                         spT[:, fc, s0 * P:s0 * P + n],
                                         start=(fc == 0), stop=(fc == C // P - 1))
                    bias = embb[:, oc:oc + 1] if has_bias else 0.0
                    nc.scalar.activation(gT[:, oc, s0 * P:s0 * P + n], ps, AF.Gelu,
                                         bias=bias)
            for rb in range(NB):
                ps = mm_ps.tile([P, 512], F32, tag="mm")
                mm_group(ps,
                         [(gT[:, fc, rb * P:(rb + 1) * P], projw[:, fc, :])
                          for fc in range(D // P)],
                         bias_row=projb[:] if has_bias else None)
                nc.scalar.activation(x[:, rb, :], ps, AF.Copy)

            # ---- layers ----
            _trunc = os.environ.get("KTRUNC", "")
            n_layers = L
            if _trunc.startswith("L"):
                n_layers = int(_trunc[1:].split(":")[0])
            _phase = _trunc.split(":")[1] if ":" in _trunc else "all"
            for l in range(n_layers):
                kb0, qb0 = l, l + 1

                wq = wts.tile([P, H // P, H], BF16, tag="wq")
                nc.sync.dma_start(out=wq[:],
                                  in_=bvw(f"wq{l}", "(p f o) -> p f o", p=P, f=H // P))
                wk = wts.tile([P, H // P, H], BF16, tag="wk")
                nc.sync.dma_start(out=wk[:],
                                  in_=bvw(f"wk{l}", "(p f o) -> p f o", p=P, f=H // P))
                wv = wts.tile([P, H // P, H], BF16, tag="wv")
                nc.sync.dma_start(out=wv[:],
                                  in_=bvw(f"wv{l}", "(p f o) -> p f o", p=P, f=H // P))
                wo = wts.tile([P, H // P, H], BF16, tag="wo")
                nc.sync.dma_start(out=wo[:],
                                  in_=bvw(f"wo{l}", "(p f o) -> p f o", p=P, f=H // P))
                if has_bias:
                    bq_b = wts.tile([P, H // P], BF16, tag="bq_b")
                    nc.sync.dma_start(out=bq_b[:],
                                      in_=bvw(f"bq{l}", "(p c) -> p c", p=P))
                    bq = wts.tile([P, H // P], F32, tag="bq")
                    nc.scalar.activation(bq[:], bq_b[:], AF.Copy)
                    bk_b = wts.tile([P, H // P], BF16, tag="bk_b")
                    nc.sync.dma_start(out=bk_b[:],
                                      in_=bvw(f"bk{l}", "(p c) -> p c", p=P))
                    bk = wts.tile([P, H // P], F32, tag="bk")
                    nc.scalar.activation(bk[:], bk_b[:], AF.Copy)
                    bv = wts.tile([1, H], BF16, tag="bv")
                    nc.sync.dma_start(out=bv[:], in_=bvw(f"bv{l}", "(a h) -> a h", a=1))
                    bo = wts.tile([1, H], BF16, tag="bo")
                    nc.sync.dma_start(out=bo[:], in_=bvw(f"bo{l}", "(a h) -> a h", a=1))
                    dnb = wts.tile([1, H], BF16, tag="dnb")
                    nc.sync.dma_start(out=dnb[:],
                                      in_=bvw(f"dnb{l}", "(a h) -> a h", a=1))
                    upb_b = wts.tile([P, INTER // P], BF16, tag="upb_b")
                    nc.sync.dma_start(out=upb_b[:],
                                      in_=bvw(f"upb{l}", "(p c) -> p c", p=P))
                    upb = wts.tile([P, INTER // P], F32, tag="upb")
                    nc.scalar.activation(upb[:], upb_b[:], AF.Copy)

                def layernorm(src_ap, dst_bf16_ap):
                    stats = small.tile([P, 6], F32, tag="stats")
                    nc.vector.bn_stats(stats[:], src_ap)
                    mv = small.tile([P, 2], F32, tag="mv")
                    nc.vector.bn_aggr(mv[:], stats[:])
                    rstd = small.tile([P, 1], F32, tag="rstd")
                    nc.scalar.activation(rstd[:], mv[:, 1:2], AF.Sqrt, bias=eps[:])
                    nc.vector.reciprocal(rstd[:], rstd[:])
                    nc.vector.tensor_scalar(dst_bf16_ap, src_ap,
                                            mv[:, 0:1], rstd[:],
                                            mybir.AluOpType.subtract,
                                            mybir.AluOpType.mult)

                def transpose128(src_bf16_ap, dst_bf16_ap):
                    # src [128, 128] -> dst [128, 128] via PE transpose
                    tp = t_ps.tile([P, P], BF16, tag="tp")
                    nc.tensor.transpose(tp[:], src_bf16_ap, ident[:])
                    nc.scalar.activation(dst_bf16_ap, tp[:], AF.Copy)

                # LN1 + h^T + v for key range
                hT = hTs.tile([P, H // P, T], BF16, tag="hT")
                vtiles = {}
                for kb in range(kb0, NB):
                    hrow = work.tile([P, H], BF16, tag="hrow")
                    layernorm(x[:, kb, :], hrow[:])
                    for fc in range(H // P):
                        transpose128(hrow[:, fc * P:(fc + 1) * P],
                                     hT[:, fc, kb * P:(kb + 1) * P])
                    ps = mm_ps.tile([P, 512], F32, tag="mm")
                    mm_group(ps,
                             [(hT[:, fc, kb * P:(kb + 1) * P], wv[:, fc, :])
                              for fc in range(H // P)],
                             bias_row=bv[:] if has_bias else None)
                    vt = vp.tile([P, NH, HD + 1], BF16, tag="v")
                    nc.scalar.activation(vt[:, :, 0:HD],
                                         ps.rearrange("p (h d) -> p h d", h=NH),
                                         AF.Copy)
                    nc.vector.memset(vt[:, :, HD:HD + 1], 1.0)
                    vtiles[kb] = vt

                if _phase == "v" and l == n_layers - 1:
                    continue
                # q^T / k^T with RoPE
                qT = qk.tile([P, H // P, T], BF16, tag="qT")
                kT = qk.tile([P, H // P, T], BF16, tag="kT")
                for (dst, w, bias_t, blk0) in (
                    (qT, wq, "bq", qb0),
                    (kT, wk, "bk", kb0),
                ):
                    for oc in range(H // P):
                        for (s0, s1) in _spans(blk0, NB):
                            n = (s1 - s0) * P
                            c0 = s0 * P
                            ps = mm_ps.tile([P, 512], F32, tag="mm", name="mmps")[:, :n]
                            for fc in range(H // P):
                                nc.tensor.matmul(ps, w[:, fc, oc * P:(oc + 1) * P],
                                                 hT[:, fc, c0:c0 + n],
                                                 start=(fc == 0),
                                                 stop=(fc == H // P - 1))
                            q0 = work.tile([P, 512], BF16, tag="q0", name="q0t")[:, :n]
                            if has_bias:
                                bt = bq if bias_t == "bq" else bk
                                nc.scalar.activation(q0, ps, AF.Copy,
                                                     bias=bt[:, oc:oc + 1])
                            else:
                                nc.scalar.activation(q0, ps, AF.Copy)
                            # rope: out = q0 * cs + rot_half(q0) * sn,
                            # rot_half via signed-permutation matmul on PE
                            rp = mm_ps.tile([P, 512], F32, tag="mm", name="rpps")[:, :n]
                            nc.tensor.matmul(rp, rotm[:], q0, start=True, stop=True)
                            t1 = work.tile([P, 512], BF16, tag="t1", name="t1t")[:, :n]
                            nc.vector.tensor_mul(t1, rp, snT[:, c0:c0 + n])
                            t2 = work.tile([P, 512], BF16, tag="t2", name="t2t")[:, :n]
                            nc.vector.tensor_mul(t2, q0, csT[:, c0:c0 + n])
                            nc.vector.tensor_add(dst[:, oc, c0:c0 + n], t1, t2)

                if _phase == "qk" and l == n_layers - 1:
                    continue
                # scores + exp per (kb), then PV/Wo for qb == kb
                estiles = {}
                for kb in range(kb0, NB):
                    qlo, qhi = max(kb, qb0), min(kb + 2, NB)
                    n = (qhi - qlo) * P
                    c0 = qlo * P
                    moff = (qlo - kb) * P
                    for h in range(NH):
                        hp0 = 64 * (h % 2)
                        hc = h // 2
                        sp = s_ps.tile([P, 2 * P], F32, tag="s", name="spt")[:, :n]
                        nc.tensor.matmul(sp,
                                         kT[hp0:hp0 + 64, hc, kb * P:(kb + 1) * P],
                                         qT[hp0:hp0 + 64, hc, c0:c0 + n],
                                         start=True, stop=True)
                        nc.vector.tensor_add(sp, sp, band[:, moff:moff + n])
                        est = es.tile([P, 2 * P], BF16, tag=f"es{h}")
                        nc.scalar.activation(est[:, moff:moff + n], sp, AF.Exp,
                                             scale=0.125, bias=kiv[:, kb:kb + 1])
                        estiles[(h, kb)] = est

                    if kb < qb0 or _phase == "scores":
                        continue
                    qb = kb
                    # PV with appended-ones denominator column
                    ops_ = [o_ps.tile([P, 4, HD + 1], F32, tag="o", name=f"opst{_g}") for _g in range(2)]
                    for h in range(NH):
                        sl = ops_[h // 4][:, h % 4, :]
                        nc.tensor.matmul(sl, estiles[(h, qb)][:, 0:P],
                                         vtiles[qb][:, h, :], start=True, stop=False)
                        nc.tensor.matmul(sl, estiles[(h, qb - 1)][:, P:2 * P],
                                         vtiles[qb - 1][:, h, :], start=False, stop=True)
                    if _phase == "pv1":
                        continue
                    den = small.tile([P, NH], F32, tag="den")
                    nc.scalar.activation(den[:, 0:4], ops_[0][:, :, HD], AF.Copy,
                                         bias=1e-20)
                    nc.scalar.activation(den[:, 4:8], ops_[1][:, :, HD], AF.Copy,
                                         bias=1e-20)
                    nc.vector.reciprocal(den[:], den[:])
                    if _phase == "pv2":
                        continue
                    osc = work.tile([P, H], BF16, tag="osc")
                    for g in range(2):
                        nc.vector.tensor_mul(
                            osc.rearrange("p (g2 h d) -> p g2 h d", g2=2, h=4)[:, g],
                            ops_[g][:, :, 0:HD],
                            den[:, g * 4:(g + 1) * 4, None].to_broadcast((P, 4, HD)))
                    if _phase == "pv":
                        continue
                    oT = work.tile([P, H // P, P], BF16, tag="oT")
                    for fc in range(H // P):
                        transpose128(osc[:, fc * P:(fc + 1) * P], oT[:, fc, :])
                    ps = mm_ps.tile([P, 512], F32, tag="mm")
                    mm_group(ps,
                             [(oT[:, fc, :], wo[:, fc, :]) for fc in range(H // P)],
                             bias_row=bo[:] if has_bias else None)
                    nc.vector.tensor_add(x[:, qb, :], ps, x[:, qb, :])

                if _phase == "attn" and l == n_layers - 1:
                    continue
                # ---- MLP ----
                h2T = hTs.tile([P, H // P, T], BF16, tag="hT")
                for qb in range(qb0, NB):
                    hrow = work.tile([P, H], BF16, tag="hrow")
                    layernorm(x[:, qb, :], hrow[:])
                    for fc in range(H // P):
                        transpose128(hrow[:, fc * P:(fc + 1) * P],
                                     h2T[:, fc, qb * P:(qb + 1) * P])

                for (s0, s1) in _spans(qb0, NB):
                    n = (s1 - s0) * P
                    c0 = s0 * P
                    it = itp.tile([P, INTER // P, 512], BF16, tag="iT")
                    for icg in range(2):
                        uw = wts.tile([P, H // P, INTER // 2], BF16, tag="upw")
                        nc.sync.dma_start(
                            out=uw[:],
                            in_=bvw(f"upw{l}", "(p f i) -> p f i", p=P, f=H // P)[
                                :, :, icg * (INTER // 2):(icg + 1) * (INTER // 2)])
                        for ic in range(INTER // 2 // P):
                            icx = icg * (INTER // 2 // P) + ic
                            ps = mm_ps.tile([P, 512], F32, tag="mm", name="mmps")[:, :n]
                            for fc in range(H // P):
                                nc.tensor.matmul(ps, uw[:, fc, ic * P:(ic + 1) * P],
                                                 h2T[:, fc, c0:c0 + n],
                                                 start=(fc == 0),
                                                 stop=(fc == H // P - 1))
                            bias = upb[:, icx:icx + 1] if has_bias else 0.0
                            nc.scalar.activation(it[:, icx, :n], ps, AF.Gelu,
                                                 bias=bias)
                    dw = [None, None]
                    for icg in range(2):
                        dw[icg] = wts.tile([P, INTER // 2 // P, H], BF16, tag="dnw",
                                           name=f"dnw{icg}")
                        nc.sync.dma_start(
                            out=dw[icg][:],
                            in_=bvw(f"dnw{l}", "(p g o) -> p g o", p=P, g=INTER // P)[
                                :, icg * (INTER // 2 // P):(icg + 1) * (INTER // 2 // P), :])
                    for qb in range(s0, s1):
                        rel = (qb - s0) * P
                        ps = mm_ps.tile([P, 512], F32, tag="mm")
                        mm_group(ps,
                                 [(it[:, icx, rel:rel + P], dw[icx // 8][:, icx % 8, :])
                                  for icx in range(INTER // P)],
                                 bias_row=dnb[:] if has_bias else None)
                        nc.vector.tensor_add(x[:, qb, :], ps, x[:, qb, :])

            # ---- output: local blocks 4..8, bf16, gathered onto every core ----
            xout = consts.tile([P, NB // 2, H], BF16, tag="xout")
            nc.scalar.activation(xout[:], x[:, NB // 2:NB, :], AF.Copy)
            nc.sync.dma_start(
                out=d_olocal.ap().rearrange("(b p) h -> p b h", p=P),
                in_=xout[:])
            nc.gpsimd.collective_compute(
                "AllGather", mybir.AluOpType.bypass,
                replica_groups=[list(range(N_CORES))],
                ins=[d_olocal.ap()], outs=[d_og.ap()])
            nc.sync.dma_start(out=d_out.ap(), in_=d_og.ap())

    nc.finalize()
    return nc


def _rope_tables():
    inv = 1.0 / (BASE ** (np.arange(0, HD, 2, dtype=np.float32) / np.float32(HD)))
    t = np.arange(T, dtype=np.float32)
    f = t[:, None] * inv[None, :]                      # [T, HD/2]
    emb = np.concatenate([f, f], axis=-1)              # [T, HD]
    return np.cos(emb).astype(np.float32), np.sin(emb).astype(np.float32)


def _bf16(x):
    return np.ascontiguousarray(np.asarray(x, np.float32)).astype(ml_dtypes.bfloat16)


def _pmajor(w, p_groups):
    """[G*128, X] row-major -> [128, G, X] p-major, raveled (bf16)."""
    w = np.asarray(w)
    g = w.shape[0] // P
    return _bf16(w.reshape(g, P, -1).transpose(1, 0, 2)).ravel()


def prepare(inputs):
    """Host-side preprocessing: returns (nc, wire0) — wire0 ships to core 0."""
    inp = {k: np.asarray(v) for k, v in inputs.items()}
    spikes = inp["spikes"].astype(np.float32)          # [B, T, C]
    spikes_mask = inp["spikes_mask"].astype(np.int32)  # [B, T]
    ts = inp["spikes_timestamp"].astype(np.int64)      # [B, T]

    # ---- fold LN gains/biases into weights host-side ----
    ln1_g, ln1_b = inp["ln1_g"].astype(np.float32), inp["ln1_b"].astype(np.float32)
    ln2_g, ln2_b = inp["ln2_g"].astype(np.float32), inp["ln2_b"].astype(np.float32)
    Wq, Wk, Wv, Wo = (inp[k].astype(np.float32) for k in ("Wq", "Wk", "Wv", "Wo"))
    upw, dnw = inp["up_w"].astype(np.float32), inp["down_w"].astype(np.float32)
    bq = inp["bq"].astype(np.float32) + np.einsum("lh,lho->lo", ln1_b, Wq)
    bk = inp["bk"].astype(np.float32) + np.einsum("lh,lho->lo", ln1_b, Wk)
    bv = inp["bv"].astype(np.float32) + np.einsum("lh,lho->lo", ln1_b, Wv)
    bo = inp["bo"].astype(np.float32)
    upb = inp["up_b"].astype(np.float32) + np.einsum("lh,lhi->li", ln2_b, upw)
    dnb = inp["down_b"].astype(np.float32)
    wq_eff = ln1_g[:, :, None] * Wq
    wk_eff = ln1_g[:, :, None] * Wk
    wv_eff = ln1_g[:, :, None] * Wv
    upw_eff = ln2_g[:, :, None] * upw

    has_bias = bool(
        np.abs(inp["embed_b"]).max() > 0 or np.abs(inp["proj_b"]).max() > 0
        or max(np.abs(a).max() for a in (bq, bk, bv, bo, upb, dnb)) > 0)

    key = has_bias
    if key not in _PROG_CACHE:
        _PROG_CACHE[key] = _build_program(has_bias)
    nc = _PROG_CACHE[key]

    blob_off, blob_elems = _blob_layout(has_bias)

    # signed permutation for rotate-half: out[m] = sign(m) * q[partner(m)]
    # (as matmul rotm.T @ q: rotm[partner(m), m] = sign(m))
    rotm_np = np.zeros((P, P), np.float32)
    for m in range(P):
        d = m % HD
        partner = m + HD // 2 if d < HD // 2 else m - HD // 2
        rotm_np[partner, m] = -1.0 if d < HD // 2 else 1.0

    blob = np.zeros(blob_elems, ml_dtypes.bfloat16)

    def put(name, arr_flat):
        off, n = blob_off[name]
        assert arr_flat.size == n, (name, arr_flat.size, n)
        blob[off:off + n] = arr_flat

    put("rotm", _bf16(rotm_np).ravel())
    # band structure in local coords, shared by all cores and key blocks:
    # col block 0 (q in same block as k): allow kc <= qc; col block 1
    # (q one block above k): allow kc >= qc.
    kc_ = np.arange(P)[:, None]
    qc_ = np.arange(P)[None, :]
    band_np = np.concatenate(
        [np.where(kc_ <= qc_, 0.0, NEG), np.where(kc_ >= qc_, 0.0, NEG)],
        axis=1).astype(np.float32)
    put("band", _bf16(band_np).ravel())
    put("embw", _pmajor(inp["embed_w"], 2))
    put("projw", _pmajor(inp["proj_w"], 2))
    for l in range(L):
        put(f"wq{l}", _pmajor(wq_eff[l], 4))
        put(f"wk{l}", _pmajor(wk_eff[l], 4))
        put(f"wv{l}", _pmajor(wv_eff[l], 4))
        put(f"wo{l}", _pmajor(Wo[l], 4))
        put(f"upw{l}", _pmajor(upw_eff[l], 4))
        put(f"dnw{l}", _pmajor(dnw[l], 16))
    if has_bias:
        put("embb", _bf16(inp["embed_b"].reshape(2, P).T).ravel())
        put("projb", _bf16(inp["proj_b"]).ravel())
        for l in range(L):
            put(f"bq{l}", _bf16(bq[l].reshape(4, P).T).ravel())
            put(f"bk{l}", _bf16(bk[l].reshape(4, P).T).ravel())
            put(f"bv{l}", _bf16(bv[l]).ravel())
            put(f"bo{l}", _bf16(bo[l]).ravel())
            put(f"upb{l}", _bf16(upb[l].reshape(16, P).T).ravel())
            put(f"dnb{l}", _bf16(dnb[l]).ravel())

    cos_t, sin_t = _rope_tables()   # [T, HD]

    pcs = []
    TH = T // 2
    for b in range(B):
        for h in range(2):
            g0 = h * TH             # global row of local row 512
            # own rows: global [g0, g0+512) live at local cols [512, 1024)
            spT_own = spikes[b, g0:g0 + TH, :].T       # [C, 512]

            ts_own = ts[b, g0:g0 + TH]
            cs_own = cos_t[ts_own][:, 0:HD // 2].T     # [32, 512]
            sn_own = sin_t[ts_own][:, 0:HD // 2].T

            # per-key additive invalid bias [kc, kb], pre-scaled by 0.125
            # (rides the Exp activation's per-partition bias column)
            gk = (np.arange(T) - TH + g0)
            kvalid = (gk >= 0) & (spikes_mask[b, np.clip(gk, 0, T - 1)] > 0)
            kiv = np.where(kvalid, 0.0, NEG * 0.125).astype(np.float32)
            kiv = kiv.reshape(NB, P).T                 # [128, NB]

            pc = np.empty(PCW, ml_dtypes.bfloat16)
            pc[_PC_SPT:_PC_SPT + P * 2 * TH] = _pmajor(spT_own, 2)
            pc[_PC_CST:_PC_CST + 32 * TH] = _bf16(cs_own).ravel()
            pc[_PC_SNT:_PC_SNT + 32 * TH] = _bf16(sn_own).ravel()
            pc[_PC_KIV:_PC_KIV + P * NB] = _bf16(kiv).ravel()
            pc[_PC_FLG:_PC_FLG + P] = ml_dtypes.bfloat16(float(h))
            pcs.append(pc)

    wire0 = np.concatenate([blob] + pcs)
    return nc, wire0


# ---------------------------------------------------------------------------
# cached-jit runner: wire ships to core 0 only; cores 1..7 get device zeros
# ---------------------------------------------------------------------------

def _get_exec(nc):
    key = id(nc)
    if key in _EXEC_CACHE:
        return _EXEC_CACHE[key]
    bass2jax.install_neuronx_cc_hook()
    partition_name = nc.partition_id_tensor.name if nc.partition_id_tensor else None
    in_names, out_names, out_avals, zero_shapes = [], [], [], []
    for alloc in nc.m.functions[0].allocations:
        if not isinstance(alloc, mybir.MemoryLocationSet):
            continue
        name = alloc.memorylocations[0].name
        if alloc.kind == "ExternalInput":
            if name != partition_name:
                in_names.append(name)
        elif alloc.kind == "ExternalOutput":
            shape = tuple(alloc.tensor_shape)
            dtype = mybir.dt.np(alloc.dtype)
            out_names.append(name)
            out_avals.append(jax.core.ShapedArray(shape, dtype))
            zero_shapes.append((shape, dtype))
    assert nc.dbg_addr is None, "runner assumes debug=False"
    assert in_names == ["wire"], in_names
    n_params = len(in_names)
    n_outs = len(out_avals)
    all_names = list(in_names) + list(out_names)
    if partition_name is not None:
        all_names.append(partition_name)
    donate = tuple(range(n_params, n_params + n_outs))

    def _body(*args):
        operands = list(args)
        if partition_name is not None:
            operands.append(bass2jax.partition_id_tensor())
        outs = bass2jax._bass_exec_p.bind(
            *operands,
            out_avals=tuple(out_avals),
            in_names=tuple(all_names),
            out_names=tuple(out_names),
            lowering_input_output_aliases=(),
            sim_require_finite=True,
            sim_require_nnan=True,
            nc=nc,
        )
        return tuple(outs)

    devices = jax.devices()[:N_CORES]
    mesh = Mesh(np.asarray(devices), ("core",))
    in_specs = (PartitionSpec("core"),) * (n_params + n_outs)
    out_specs = (PartitionSpec("core"),) * n_outs
    sharded = jax.jit(
        shard_map(_body, mesh=mesh, in_specs=in_specs, out_specs=out_specs,
                  check_rep=False),
        donate_argnums=donate, keep_unused=True)

    core_sharding = NamedSharding(mesh, PartitionSpec("core"))
    zeros_out = jax.jit(
        lambda: tuple(jnp.zeros((N_CORES * s[0], *s[1:]), d)
                      for s, d in zero_shapes),
        out_shardings=(core_sharding,) * n_outs)

    st = dict(sharded=sharded, devices=devices, core_sharding=core_sharding,
              zeros_out=zeros_out, out_names=out_names, zero_dev=None)
    _EXEC_CACHE[key] = st
    return st


def run_model(nc, wire0):
    """One full inference: ship wire0 to core 0, run, fetch output [B, T, H]."""
    st = _get_exec(nc)
    devices = st["devices"]
    if st["zero_dev"] is None:
        # persistent zero padding shards for cores 1..7 (inputs, not donated)
        st["zero_dev"] = [
            jax.jit(lambda: jnp.zeros(wire0.shape, wire0.dtype),
                    out_shardings=SingleDeviceSharding(d))()
            for d in devices[1:]]
    shard0 = jax.device_put(wire0, devices[0])
    shards = [shard0] + st["zero_dev"]
    gshape = (N_CORES * wire0.shape[0],) + wire0.shape[1:]
    wire_g = jax.make_array_from_single_device_arrays(
        gshape, st["core_sharding"], shards)
    zouts = st["zeros_out"]()
    out_arrs = st["sharded"](wire_g, *zouts)
    # output was AllGathered on-device: every core holds the full result, so
    # fetch only core 0's shard (one d2h transfer)
    res = np.asarray(out_arrs[0].addressable_shards[0].data)
    res = res.reshape(N_CORES, T // 2, H)
    out = np.empty((B, T, H), np.float32)
    for b in range(B):
        for h in range(2):
            out[b, h * (T // 2):(h + 1) * (T // 2), :] = res[b * 2 + h]
    return out


def kernel(**inputs):
    nc, wire0 = prepare(inputs)
    return run_model(nc, wire0)
